# revision 1
# baseline (speedup 1.0000x reference)
"""Baichuan transformer layer on 8 Trainium2 NeuronCores, tensor-parallel.

Sharding: heads (32 -> 4/core) and MLP intermediate (11008 -> ~1376/core,
padded to 1408) are split across 8 cores. W_pack/gate/up sharded column-wise,
o_proj/down row-wise. bf16 ReduceScatter after o_proj (sequence-sharded
fp32 residual + RMSNorm), AllGather of the normed bf16 activations,
bf16 ReduceScatter after down_proj. Final output is assembled from
per-core sequence shards.

Dataflow keeps activations transposed ([hidden, seq]) for all matmuls; the
residual stream stays natural [seq, hidden] in fp32.
"""

import math
import os
import sys

sys.path.insert(0, "/opt/trn_rl_repo")

import ml_dtypes
import numpy as np

import concourse.bass as bass
import concourse.tile as tile
from concourse import bacc, mybir
from concourse.masks import make_identity

P = 128
S = 2048
H = 4096
NKC = H // P            # 32 hidden chunks
NH_LOC = 4              # heads per core
DH = 128
QKV_LOC = NH_LOC * DH   # 512
I_LOC = 1408            # padded local intermediate (11 * 128)
NIT = I_LOC // P        # 11
NST = S // P            # 16 seq tiles
NCH = 4                 # collective chunks
CHS = S // NCH          # 512 tokens per chunk
SHR = CHS // 8          # 64 rows per rank shard per chunk
EPS = 1e-6
SCALE = 1.0 / math.sqrt(DH)
BF = mybir.dt.bfloat16
F32 = mybir.dt.float32

COLL_DT = mybir.dt.bfloat16  # collective dtype (partials; residual math stays fp32)

_CACHE = {}


def _build():
    nc = bacc.Bacc("TRN2", target_bir_lowering=False, debug=False, num_devices=8)

    hiddent = nc.dram_tensor("hiddent", [H, S], F32, kind="ExternalInput")
    hidshard = nc.dram_tensor("hidshard", [NCH, SHR, H], F32, kind="ExternalInput")
    maskt = nc.dram_tensor("maskt", [NST, P, P], F32, kind="ExternalInput")
    wpack = nc.dram_tensor("wpack", [H, 3 * QKV_LOC], BF, kind="ExternalInput")
    oproj = nc.dram_tensor("oproj", [QKV_LOC, H], BF, kind="ExternalInput")
    gatew = nc.dram_tensor("gatew", [H, I_LOC], BF, kind="ExternalInput")
    upw = nc.dram_tensor("upw", [H, I_LOC], BF, kind="ExternalInput")
    downw = nc.dram_tensor("downw", [I_LOC, H], BF, kind="ExternalInput")
    ln1 = nc.dram_tensor("ln1", [P, NKC], F32, kind="ExternalInput")
    ln2 = nc.dram_tensor("ln2", [1, H], F32, kind="ExternalInput")
    out = nc.dram_tensor("out", [NCH, SHR, H], F32, kind="ExternalOutput")

    RG = [list(range(8))]
    MUL = mybir.AluOpType.mult
    ADD = mybir.AluOpType.add
    AF = mybir.ActivationFunctionType

    with tile.TileContext(nc) as tc:
      with tc.tile_pool(name="const", bufs=1) as cp, \
           tc.tile_pool(name="dram", bufs=1, space="DRAM") as dp:
        # ---- tiny constants (live whole kernel, ~1KB/partition) ----
        ln1sb = cp.tile([P, NKC], F32)
        nc.sync.dma_start(ln1sb[:], ln1[:])
        ident = cp.tile([P, P], F32)
        make_identity(nc, ident[:])
        ones_bf = cp.tile([P, 1], BF)
        nc.vector.memset(ones_bf[:], 1.0)
        epssb = cp.tile([P, 1], F32)
        nc.vector.memset(epssb[:], EPS)
        rsq_pcol = cp.tile([P, NST], F32)

        # dram scratch
        rsq_d = dp.tile([1, S], F32)
        rs1_in = [dp.tile([CHS, H], COLL_DT, name=f"rs1_in{c}")
                  for c in range(NCH)]
        rs1_out = dp.tile([NCH, SHR, H], COLL_DT)
        ag_in = dp.tile([NCH, H, SHR], BF)
        ag_out = [dp.tile([8 * H, SHR], BF, addr_space="Shared", name=f"ag_out{c}")
                  for c in range(NCH)]
        rs2_in = [dp.tile([CHS, H], COLL_DT, name=f"rs2_in{c}")
                  for c in range(NCH)]
        rs2_out = dp.tile([NCH, SHR, H], COLL_DT)
        had_d = dp.tile([NIT, P, S], BF)
        rec_d = dp.tile([NH_LOC, NST, P], F32)
        v_d = dp.tile([NST, P, QKV_LOC], BF)

        # long-lived pools with manual open/close (two-sided allocator)
        atp_cm = tc.tile_pool(name="atp", bufs=1)          # left: attnT p1-p4
        atp = atp_cm.__enter__()
        attnT = atp.tile([P, NH_LOC, S], BF)
        ht_cm = tc.tile_pool(name="ht", bufs=1)            # left: p1-p2
        htp = ht_cm.__enter__()
        ht = htp.tile([P, NKC, S], BF)

        # ==== phase 1: hiddenT load, sumsq, cast*ln1 ====
        with tc.tile_pool(name="hin", bufs=4) as hinp, \
             tc.tile_pool(name="sqp", bufs=2) as sqp, \
             tc.tile_pool(name="ssps", bufs=1, space="PSUM") as ssp, \
             tc.tile_pool(name="smal", bufs=1) as smp:
            ss = ssp.tile([1, S], F32)
            for k in range(NKC):
                hf = hinp.tile([P, S], F32, tag="hf")
                nc.sync.dma_start(hf[:], hiddent[k * P:(k + 1) * P, :])
                sq = sqp.tile([P, S], BF, tag="sq")
                nc.scalar.activation(sq[:], hf[:], AF.Square)
                for j in range(4):
                    nc.tensor.matmul(
                        ss[:, j * 512:(j + 1) * 512], ones_bf[:],
                        sq[:, j * 512:(j + 1) * 512],
                        start=(k == 0), stop=(k == NKC - 1))
                nc.vector.tensor_tensor(
                    ht[:, k, :], hf[:],
                    ln1sb[:, k:k + 1].to_broadcast((P, S)), MUL)
            # rsq = 1/sqrt(mean + eps)
            std = smp.tile([1, S], F32, tag="std")
            nc.scalar.activation(std[:], ss[:], AF.Sqrt,
                                 bias=epssb[:1, :], scale=1.0 / H)
            rsq = smp.tile([1, S], F32, tag="rsq")
            nc.vector.reciprocal(rsq[:], std[:])
            nc.sync.dma_start(rsq_d[:], rsq[:])
            nc.sync.dma_start(
                rsq_pcol[:], rsq_d.rearrange("o (n p) -> p (o n)", p=P))

        # right side: qkv outputs, live p2-p3
        qkv_cm = tc.tile_pool(name="qkv", bufs=1, side="right")
        qkvp = qkv_cm.__enter__()
        qT = qkvp.tile([P, NH_LOC, S], BF)
        kT = qkvp.tile([P, NH_LOC, S], BF)
        rsq_bc = qkvp.tile([P, S], BF)
        nc.gpsimd.dma_start(rsq_bc[:], rsq_d[:].to_broadcast((P, S)))

        # ==== phase 2: QKV projections ====
        with tc.tile_pool(name="wst", bufs=2) as wsp, \
             tc.tile_pool(name="qps", bufs=1, space="PSUM") as qpsp:
            for part in range(2):       # 0 = q, 1 = k
                dst = qT if part == 0 else kT
                for h in range(NH_LOC):
                    wcol = wsp.tile([P, NKC, P], BF, tag="wcol")
                    col0 = part * QKV_LOC + h * DH
                    nc.scalar.dma_start(
                        wcol[:],
                        wpack.rearrange("(k p) c -> p k c", p=P)
                        [:, :, col0:col0 + DH])
                    ps = [qpsp.tile([P, 512], F32, tag=f"qk{j}", name=f"qk{j}")
                          for j in range(4)]
                    for k in range(NKC):
                        for j in range(4):
                            nc.tensor.matmul(
                                ps[j][:], wcol[:, k, :],
                                ht[:, k, j * 512:(j + 1) * 512],
                                start=(k == 0), stop=(k == NKC - 1))
                    for j in range(4):
                        nc.vector.tensor_tensor(
                            dst[:, h, j * 512:(j + 1) * 512], ps[j][:],
                            rsq_bc[:, j * 512:(j + 1) * 512], MUL)
            # v in natural [s, d] layout (lhsT = hT chunk), staged to DRAM
            for vg in range(2):
                ps = [qpsp.tile([P, 512], F32, tag=f"qk{j}", name=f"vq{j}")
                      for j in range(4)] + \
                     [qpsp.tile([P, 512], F32, tag=f"v{j}", name=f"v{j}")
                      for j in range(4)]
                for k in range(NKC):
                    wv = wsp.tile([P, QKV_LOC], BF, tag="wv")
                    nc.scalar.dma_start(
                        wv[:], wpack[k * P:(k + 1) * P,
                                     2 * QKV_LOC:3 * QKV_LOC])
                    for sti in range(8):
                        st = vg * 8 + sti
                        nc.tensor.matmul(
                            ps[sti][:], ht[:, k, st * P:(st + 1) * P],
                            wv[:], start=(k == 0), stop=(k == NKC - 1))
                for sti in range(8):
                    st = vg * 8 + sti
                    vstg = wsp.tile([P, QKV_LOC], BF, tag="vstg")
                    nc.scalar.activation(
                        vstg[:], ps[sti][:], AF.Copy,
                        scale=rsq_pcol[:, st:st + 1])
                    nc.sync.dma_start(v_d[st], vstg[:])

        ht_cm.__exit__(None, None, None)   # free 128KB/part

        # prefetch o_proj weights during attention
        opj_cm = tc.tile_pool(name="opj", bufs=1)
        opp = opj_cm.__enter__()
        ow = opp.tile([P, NH_LOC, H], BF)
        for h in range(NH_LOC):
            nc.scalar.dma_start(ow[:, h, :], oproj[h * P:(h + 1) * P, :])

        # ==== phase 3: attention ====
        with tc.tile_pool(name="msk", bufs=1) as mkp, \
             tc.tile_pool(name="probs", bufs=6) as prp, \
             tc.tile_pool(name="vh", bufs=2) as vhp, \
             tc.tile_pool(name="scps", bufs=2, space="PSUM") as scp, \
             tc.tile_pool(name="atps", bufs=1, space="PSUM") as apsp, \
             tc.tile_pool(name="attmisc", bufs=2) as amp:
            masksb = mkp.tile([P, NST, P], F32)
            nc.sync.dma_start(masksb[:], maskt.rearrange("n k q -> k n q"))
            v_r = v_d.rearrange("st p c -> p st c")
            for h in range(NH_LOC):
                vh = vhp.tile([P, NST, DH], BF, tag="vh")
                nc.sync.dma_start(vh[:], v_r[:, :, h * DH:(h + 1) * DH])
                aps = apsp.tile([P, S], F32, tag="aps", name="aps")
                sps = apsp.tile([P, NST], F32, tag="sps", name="sps")
                for kb in range(NST):
                    q0 = kb * P
                    pt = prp.tile([P, S], BF, tag="probs", name="pt")
                    bnds = []
                    a = q0
                    while a < S:
                        b = min((a // 512 + 1) * 512, S)
                        bnds.append((a, b))
                        a = b
                    for (a, b) in bnds:
                        sc = scp.tile([P, 512], F32, tag="sc", name="sc")
                        nc.tensor.matmul(
                            sc[:, :b - a], kT[:, h, q0:q0 + P],
                            qT[:, h, a:b], start=True, stop=True)
                        if a == q0:
                            nc.vector.tensor_tensor(
                                sc[:, :P], sc[:, :P], masksb[:, kb, :], ADD)
                        nc.scalar.activation(
                            pt[:, a:b], sc[:, :b - a], AF.Exp, scale=SCALE)
                    for (a, b) in bnds:
                        nc.tensor.matmul(
                            aps[:, a:b], vh[:, kb, :], pt[:, a:b],
                            start=(kb == 0), stop=(kb == (b - 1) // P))
                    for qb in range(kb, NST):
                        # single bank shared by 16 accumulation chains:
                        # only the very first matmul may clear the bank
                        nc.tensor.matmul(
                            sps[:, qb:qb + 1], pt[:, qb * P:(qb + 1) * P],
                            ones_bf[:], start=(kb == 0 and qb == 0),
                            stop=(kb == qb), skip_group_check=True)
                rec = amp.tile([P, NST], F32, tag="rec")
                nc.vector.reciprocal(rec[:], sps[:])
                rtp = apsp.tile([NST, P], F32, tag="rtp", name="rtp")
                nc.tensor.transpose(rtp[:], rec[:], ident[:])
                rts = amp.tile([NST, P], F32, tag="rts")
                nc.scalar.copy(rts[:], rtp[:])
                nc.sync.dma_start(rec_d[h], rts[:])
                rbc = amp.tile([P, S], F32, tag="rbc")
                nc.gpsimd.dma_start(
                    rbc[:],
                    rec_d[h].rearrange("a b -> (a b)")[None, :]
                    .to_broadcast((P, S)))
                nc.vector.tensor_tensor(attnT[:, h, :], aps[:], rbc[:], MUL)

        qkv_cm.__exit__(None, None, None)

        # residual stream shards, live to the end (right side)
        h2_cm = tc.tile_pool(name="h2", bufs=1, side="right")
        h2p = h2_cm.__enter__()
        h2pk = [h2p.tile([P, H], F32, tag=f"h2_{j}", name=f"h2_{j}")
                for j in range(NCH // 2)]

        def h2sl(c):
            return h2pk[c // 2][(c % 2) * SHR:(c % 2) * SHR + SHR, :]

        # ==== phase 4: o_proj + per-chunk [RS1 -> norm -> AG] ====
        with tc.tile_pool(name="ops", bufs=1, space="PSUM") as opsp, \
             tc.tile_pool(name="ost", bufs=3) as ostp, \
             tc.tile_pool(name="chk", bufs=1) as chp:
            ln2bc = chp.tile([P, H], BF, tag="ln2bc")
            nc.gpsimd.dma_start(ln2bc[:], ln2[:].to_broadcast((P, H)))
            for st in range(NST):
                ps8 = [opsp.tile([P, 512], F32, tag=f"o{j}", name=f"o{j}")
                       for j in range(8)]
                for h in range(NH_LOC):
                    for j in range(8):
                        nc.tensor.matmul(
                            ps8[j][:], attnT[:, h, st * P:(st + 1) * P],
                            ow[:, h, j * 512:(j + 1) * 512],
                            start=(h == 0), stop=(h == NH_LOC - 1))
                osb = ostp.tile([P, H], COLL_DT, tag="osb")
                for j in range(8):
                    if j % 2 == 0:
                        nc.vector.tensor_copy(
                            osb[:, j * 512:(j + 1) * 512], ps8[j][:])
                    else:
                        nc.scalar.copy(
                            osb[:, j * 512:(j + 1) * 512], ps8[j][:])
                nc.sync.dma_start(
                    rs1_in[st // 4][(st % 4) * P:(st % 4 + 1) * P, :], osb[:])
                if st % 4 == 3:
                    c = st // 4
                    nc.gpsimd.collective_compute(
                        "ReduceScatter", ADD, replica_groups=RG,
                        ins=[rs1_in[c][:].opt()],
                        outs=[rs1_out[c].opt()])
            # per-chunk residual + rmsnorm + AllGather, emitted after the
            # o_proj loop so their RS1-waits don't block engine queues
            for c in range(NCH):
                b = (c % 2) * SHR
                h2c = h2sl(c)
                nc.sync.dma_start(h2c, hidshard[c])
                tmp = chp.tile([P, H], COLL_DT, tag="tmp")
                nc.sync.dma_start(tmp[b:b + SHR, :], rs1_out[c])
                nc.vector.tensor_tensor(h2c, h2c, tmp[b:b + SHR, :], ADD)
                sq2 = chp.tile([P, H], BF, tag="msh", name="sq2")
                nc.scalar.activation(sq2[b:b + SHR, :], h2c, AF.Square)
                var = chp.tile([P, 1], F32, tag="var")
                nc.vector.reduce_sum(var[b:b + SHR, :], sq2[b:b + SHR, :],
                                     axis=mybir.AxisListType.X)
                std2 = chp.tile([P, 1], F32, tag="std2")
                nc.scalar.activation(std2[b:b + SHR, :], var[b:b + SHR, :],
                                     AF.Sqrt, bias=epssb[b:b + SHR, :],
                                     scale=1.0 / H)
                rst = chp.tile([P, 1], F32, tag="rst")
                nc.vector.reciprocal(rst[b:b + SHR, :], std2[b:b + SHR, :])
                mtm = chp.tile([P, H], BF, tag="mtm")
                nc.scalar.activation(mtm[b:b + SHR, :], h2c, AF.Copy,
                                     scale=rst[b:b + SHR, :])
                msh = chp.tile([P, H], BF, tag="msh")
                nc.vector.tensor_tensor(msh[b:b + SHR, :], mtm[b:b + SHR, :],
                                        ln2bc[b:b + SHR, :], MUL)
                mts = chp.tile([P, NKC, SHR], BF, tag="mts")
                nc.sync.dma_start_transpose(mts[:], msh[b:b + SHR, :])
                nc.sync.dma_start(
                    ag_in[c].rearrange("(ks p) n -> p ks n", p=P), mts[:])
                nc.gpsimd.collective_compute(
                    "AllGather", mybir.AluOpType.bypass, replica_groups=RG,
                    ins=[ag_in[c].opt()], outs=[ag_out[c].opt()])

        opj_cm.__exit__(None, None, None)
        atp_cm.__exit__(None, None, None)

        mt_cm = tc.tile_pool(name="mt", bufs=1)
        mtp = mt_cm.__enter__()
        mT = [mtp.tile([P, NKC, CHS], BF, name=f"mT{c}") for c in range(NCH)]

        # ==== phase 6: gate/up + silu (chunk-outer: overlap with AG pipeline) ====
        with tc.tile_pool(name="gst", bufs=2) as gsp, \
             tc.tile_pool(name="gwa", bufs=2) as gwap, \
             tc.tile_pool(name="gwb", bufs=2) as gwbp, \
             tc.tile_pool(name="gps", bufs=1, space="PSUM") as gpsp:
            import bass_rust as _br
            gw_r = gatew.rearrange("(k p) c -> p k c", p=P)
            uw_r = upw.rearrange("(k p) c -> p k c", p=P)
            prev_mm = None
            for c in range(NCH):
                c0 = c * CHS
                for r in range(8):
                    gi = nc.sync.dma_start(
                        mT[c][:, :, r * SHR:(r + 1) * SHR],
                        ag_out[c][r * H:(r + 1) * H, :]
                        .rearrange("(ks p) n -> p ks n", p=P))
                    if prev_mm is not None:
                        # ordering-only edge: keep chunk c's gathers from being
                        # scheduled ahead of chunk c-1's compute (their AG-gated
                        # completion otherwise pollutes the DMA-lane counters
                        # that earlier work waits on)
                        _br.add_dep_helper(gi.ins, prev_mm.ins, sync=False,
                                           reason="order gathers after prev mlp")
                for i in range(NIT):
                    gcol = gwap.tile([P, NKC, P], BF, tag="gcol")
                    nc.scalar.dma_start(gcol[:], gw_r[:, :, i * P:(i + 1) * P])
                    ucol = gwbp.tile([P, NKC, P], BF, tag="ucol")
                    nc.scalar.dma_start(ucol[:], uw_r[:, :, i * P:(i + 1) * P])
                    gp = gpsp.tile([P, 512], F32, tag=f"g{i % 4}", name="gp")
                    up = gpsp.tile([P, 512], F32, tag=f"u{i % 4}", name="up")
                    for k in range(NKC):
                        mmg = nc.tensor.matmul(
                            gp[:], gcol[:, k, :], mT[c][:, k, :],
                            start=(k == 0), stop=(k == NKC - 1))
                        if i == 0 and k == 0:
                            prev_mm = mmg
                        nc.tensor.matmul(
                            up[:], ucol[:, k, :], mT[c][:, k, :],
                            start=(k == 0), stop=(k == NKC - 1))
                    gs = gsp.tile([P, CHS], BF, tag="gs")
                    us = gsp.tile([P, CHS], BF, tag="us")
                    nc.scalar.activation(gs[:], gp[:], AF.Silu)
                    nc.vector.tensor_copy(us[:], up[:])
                    hadt = gsp.tile([P, CHS], BF, tag="hadt")
                    nc.vector.tensor_tensor(hadt[:], gs[:], us[:], MUL)
                    nc.sync.dma_start(had_d[i][:, c0:c0 + CHS], hadt[:])

        mt_cm.__exit__(None, None, None)

        # ==== phase 7: down proj + RS2 ====
        with tc.tile_pool(name="dw", bufs=1) as dwp, \
             tc.tile_pool(name="dst", bufs=2) as dsp, \
             tc.tile_pool(name="hst", bufs=4) as hsp, \
             tc.tile_pool(name="dps", bufs=1, space="PSUM") as dpsp:
            dw = dwp.tile([P, NIT, H], BF)
            for i in range(NIT):
                nc.scalar.dma_start(dw[:, i, :], downw[i * P:(i + 1) * P, :])
            had_r = had_d.rearrange("i p s -> p i s")
            for st in range(NST):
                hads = hsp.tile([P, NIT, P], BF, tag="hads")
                nc.sync.dma_start(hads[:], had_r[:, :, st * P:(st + 1) * P])
                ps8 = [dpsp.tile([P, 512], F32, tag=f"d{j}", name=f"d{j}")
                       for j in range(8)]
                for i in range(NIT):
                    for j in range(8):
                        nc.tensor.matmul(
                            ps8[j][:], hads[:, i, :],
                            dw[:, i, j * 512:(j + 1) * 512],
                            start=(i == 0), stop=(i == NIT - 1))
                dsb = dsp.tile([P, H], COLL_DT, tag="dsb")
                for j in range(8):
                    if j % 2 == 0:
                        nc.vector.tensor_copy(
                            dsb[:, j * 512:(j + 1) * 512], ps8[j][:])
                    else:
                        nc.scalar.copy(
                            dsb[:, j * 512:(j + 1) * 512], ps8[j][:])
                nc.sync.dma_start(
                    rs2_in[st // 4][(st % 4) * P:(st % 4 + 1) * P, :], dsb[:])
                if st % 4 == 3:
                    c = st // 4
                    nc.gpsimd.collective_compute(
                        "ReduceScatter", ADD, replica_groups=RG,
                        ins=[rs2_in[c][:].opt()],
                        outs=[rs2_out[c].opt()])
            # ==== phase 8: final residual ====
            with tc.tile_pool(name="fin", bufs=1) as fpp:
                for c in range(NCH):
                    b = (c % 2) * SHR
                    f1 = fpp.tile([P, H], COLL_DT, tag="f1")
                    nc.sync.dma_start(f1[b:b + SHR, :], rs2_out[c])
                    fo = fpp.tile([P, H], F32, tag="fo")
                    nc.vector.tensor_tensor(fo[b:b + SHR, :], f1[b:b + SHR, :],
                                            h2sl(c), ADD)
                    nc.sync.dma_start(out[c], fo[b:b + SHR, :])

        h2_cm.__exit__(None, None, None)

    nc.finalize()
    return nc


def _prep_inputs(hidden_states, attention_mask, W_pack, o_proj, gate_w, up_w,
                 down_w, ln1_w, ln2_w):
    """Slice/layout full inputs into 8 per-core input dicts."""
    hs = np.ascontiguousarray(np.asarray(hidden_states, dtype=np.float32)[0])
    hiddent = np.ascontiguousarray(hs.T)                      # [H, S]
    mask = np.asarray(attention_mask, dtype=np.float32)[0, 0]  # [S, S]
    masktd = np.stack([
        np.ascontiguousarray(mask[b * P:(b + 1) * P, b * P:(b + 1) * P].T)
        for b in range(NST)])                                  # [NST, P, P]
    W_pack = np.asarray(W_pack, dtype=np.float32)
    o_proj = np.asarray(o_proj, dtype=np.float32)
    gate_w = np.asarray(gate_w, dtype=np.float32)
    up_w = np.asarray(up_w, dtype=np.float32)
    down_w = np.asarray(down_w, dtype=np.float32)
    ln1 = np.ascontiguousarray(
        np.asarray(ln1_w, dtype=np.float32).reshape(NKC, P).T)  # [P, NKC]
    ln2 = np.asarray(ln2_w, dtype=np.float32).reshape(1, H)

    # intermediate split: 6 cores x 1408 + 2 cores x 1280 (padded to 1408)
    i_sizes = [1408] * 6 + [1280] * 2
    i_offs = np.cumsum([0] + i_sizes)

    in_maps = []
    for r in range(8):
        q0 = r * QKV_LOC
        wp = np.concatenate([
            W_pack[:, q0:q0 + QKV_LOC],
            W_pack[:, H + q0:H + q0 + QKV_LOC],
            W_pack[:, 2 * H + q0:2 * H + q0 + QKV_LOC]], axis=1)
        opl = o_proj[q0:q0 + QKV_LOC, :]
        io0, io1 = i_offs[r], i_offs[r + 1]
        isz = io1 - io0
        gl = np.zeros((H, I_LOC), np.float32)
        gl[:, :isz] = gate_w[:, io0:io1]
        ul = np.zeros((H, I_LOC), np.float32)
        ul[:, :isz] = up_w[:, io0:io1]
        dl = np.zeros((I_LOC, H), np.float32)
        dl[:isz, :] = down_w[io0:io1, :]
        hsh = np.stack([
            hs[c * CHS + r * SHR: c * CHS + (r + 1) * SHR, :]
            for c in range(NCH)])                              # [NCH, SHR, H]
        bf = ml_dtypes.bfloat16
        in_maps.append({
            "hiddent": hiddent,
            "hidshard": np.ascontiguousarray(hsh),
            "maskt": masktd,
            "wpack": np.ascontiguousarray(wp).astype(bf),
            "oproj": np.ascontiguousarray(opl).astype(bf),
            "gatew": gl.astype(bf),
            "upw": ul.astype(bf),
            "downw": dl.astype(bf),
            "ln1": ln1,
            "ln2": ln2,
        })
    return in_maps


def _assemble(results):
    """results[r]['out'] is [NCH, SHR, H]; reassemble [1, S, H]."""
    full = np.empty((S, H), np.float32)
    for r in range(8):
        o = results[r]["out"]
        for c in range(NCH):
            full[c * CHS + r * SHR: c * CHS + (r + 1) * SHR, :] = o[c]
    return full[None]


def _get_nc():
    if "nc" not in _CACHE:
        _CACHE["nc"] = _build()
    return _CACHE["nc"]


def kernel(**inputs):
    from concourse.bass_utils import run_bass_kernel_spmd
    nc = _get_nc()
    in_maps = _prep_inputs(**inputs)
    res = run_bass_kernel_spmd(nc, in_maps, core_ids=list(range(8)))
    return _assemble(res.results)


if __name__ == "__main__":
    rng = np.random.RandomState(0)
    ins = {
        "hidden_states": rng.randn(1, S, H).astype(np.float32),
        "attention_mask": np.where(
            np.tril(np.ones((S, S), bool)), 0.0,
            np.finfo(np.float32).min)[None, None].astype(np.float32),
        "W_pack": rng.randn(H, 3 * H).astype(np.float32) * 0.02,
        "o_proj": rng.randn(H, H).astype(np.float32) * 0.02,
        "gate_w": rng.randn(H, 11008).astype(np.float32) * 0.02,
        "up_w": rng.randn(H, 11008).astype(np.float32) * 0.02,
        "down_w": rng.randn(11008, H).astype(np.float32) * 0.02,
        "ln1_w": np.ones(H, np.float32),
        "ln2_w": np.ones(H, np.float32),
    }
    out = kernel(**ins)
    print("kernel output", out.shape, out.dtype, float(np.abs(out).mean()))



# revision 45
# speedup vs baseline: 1.0589x; 1.0589x over previous
"""Baichuan transformer layer on 8 Trainium2 NeuronCores, tensor-parallel.

Sharding: heads (32 -> 4/core) and MLP intermediate (11008 -> 1376/core,
padded to 1536) split across 8 cores. W_pack/gate/up sharded column-wise,
o_proj/down row-wise. bf16 ReduceScatter after o_proj (sequence-sharded
fp32 residual + RMSNorm), bf16 AllGather of the normed activations,
bf16 ReduceScatter after down_proj.

All heavy GEMMs (QKV, o_proj, gate/up, down) run as hi/lo-split fp8
DoubleRow matmuls: X @ W ~= Xhi@Whi + Xlo@Whi + Xhi@Wlo with hi parts in
e4m3 and lo parts in e5m2 (3 DoubleRow matmuls at 0.5 cyc/row replace 2
bf16 matmuls at 1 cyc/row per 256-deep contraction slice, at better-than-
bf16 accuracy). Attention (scores/softmax/PV) stays bf16. Weights are
pre-scaled by 64 on the host; the 1/64 descale folds into existing
epilogue scales.
"""

import math
import os
import sys

sys.path.insert(0, "/opt/trn_rl_repo")

import ml_dtypes
import numpy as np

import concourse.bass as bass
import concourse.tile as tile
from concourse import bacc, mybir
from concourse.masks import make_identity

P = 128
S = 2048
H = 4096
NKC = H // P            # 32 hidden chunks
NKP = NKC // 2          # 16 hidden pair-chunks
NH_LOC = 4              # heads per core
DH = 128
QKV_LOC = NH_LOC * DH   # 512
I_LOC = 1536            # padded local intermediate (12 * 128)
I_REAL = 11008 // 8     # 1376
NIT = I_LOC // P        # 12
NIP = NIT // 2          # 6
NST = S // P            # 16 seq tiles
NCH = 4                 # collective chunks
CHS = S // NCH          # 512 tokens per chunk
SHR = CHS // 8          # 64 rows per rank shard per chunk
EPS = 1e-6
SCALE = 1.0 / math.sqrt(DH)
WS = 64.0               # host-side weight scale before fp8 cast
BF = mybir.dt.bfloat16
F32 = mybir.dt.float32
F8 = mybir.dt.float8e4
# lo parts in e5m2: the ~2% split residuals are NORMAL-range in e5m2
# (min normal 2^-14) but subnormal in e4m3 (min normal 2^-6), and HW
# DoubleRow mishandles subnormal e4m3 inputs (e4m3-lo runs measured ~2x
# worse than e5m2-lo); e5m2 x e4m3 mixed DR probed bit-exact on HW
F8L = mybir.dt.float8e5

COLL_DT = mybir.dt.bfloat16
DR = mybir.MatmulPerfMode.DoubleRow

_CACHE = {}


def _build():
    nc = bacc.Bacc("TRN2", target_bir_lowering=False, debug=False, num_devices=8)

    hiddent = nc.dram_tensor("hiddent", [H, S], BF, kind="ExternalInput")
    hidshard = nc.dram_tensor("hidshard", [NCH, SHR, H], F32, kind="ExternalInput")
    maskt = nc.dram_tensor("maskt", [NST, P, P], F32, kind="ExternalInput")
    # [part(q/k), head, p, k, dh]
    wqk8h = nc.dram_tensor("wqk8h", [2, NH_LOC, P, NKC, DH], F8, kind="ExternalInput")
    wqk8l = nc.dram_tensor("wqk8l", [2, NH_LOC, P, NKC, DH], F8L, kind="ExternalInput")
    # [k-pair, p, 2, 512]
    wv8h = nc.dram_tensor("wv8h", [NKP, P, 2, QKV_LOC], F8, kind="ExternalInput")
    wv8l = nc.dram_tensor("wv8l", [NKP, P, 2, QKV_LOC], F8L, kind="ExternalInput")
    ow8h = nc.dram_tensor("ow8h", [P, NH_LOC, H], F8, kind="ExternalInput")
    ow8l = nc.dram_tensor("ow8l", [P, NH_LOC, H], F8L, kind="ExternalInput")
    g8h = nc.dram_tensor("g8h", [NIT, P, NKC, P], F8, kind="ExternalInput")
    g8l = nc.dram_tensor("g8l", [NIT, P, NKC, P], F8L, kind="ExternalInput")
    u8h = nc.dram_tensor("u8h", [NIT, P, NKC, P], F8, kind="ExternalInput")
    u8l = nc.dram_tensor("u8l", [NIT, P, NKC, P], F8L, kind="ExternalInput")
    d8h = nc.dram_tensor("d8h", [8, P, NIT, 512], F8, kind="ExternalInput")
    d8l = nc.dram_tensor("d8l", [8, P, NIT, 512], F8L, kind="ExternalInput")
    ln1 = nc.dram_tensor("ln1", [P, NKC], F32, kind="ExternalInput")
    ln2 = nc.dram_tensor("ln2", [1, H], F32, kind="ExternalInput")
    out = nc.dram_tensor("out", [NCH, SHR, H], F32, kind="ExternalOutput")

    RG = [list(range(8))]
    MUL = mybir.AluOpType.mult
    ADD = mybir.AluOpType.add
    SUB = mybir.AluOpType.subtract
    AF = mybir.ActivationFunctionType

    with tile.TileContext(nc) as tc:
      with tc.tile_pool(name="const", bufs=1) as cp, \
           tc.tile_pool(name="dram", bufs=1, space="DRAM") as dp:
        # ---- tiny constants ----
        ln1sb = cp.tile([P, NKC], F32)
        nc.sync.dma_start(ln1sb[:], ln1[:])
        ident = cp.tile([P, P], F32)
        make_identity(nc, ident[:])
        ident_bf = cp.tile([P, SHR], BF)
        make_identity(nc, ident_bf[:SHR, :])
        nc.sync.dma_start(ident_bf[SHR:2 * SHR, :], ident_bf[:SHR, :])
        ones_bf = cp.tile([P, 1], BF)
        nc.vector.memset(ones_bf[:], 1.0)
        # pair-step of the stationary AP must be even and 16B-aligned for
        # DoubleRow ldweights, so pad the ones column to 16 bytes
        ones8 = cp.tile([P, 2, 16], F8)
        nc.vector.memset(ones8[:], 1.0)
        epssb = cp.tile([P, 1], F32)
        nc.vector.memset(epssb[:], EPS)
        rsq_pcol = cp.tile([P, NST], F32)

        # dram scratch
        rsq_d = dp.tile([1, S], F32)
        rec_d = dp.tile([NH_LOC, NST, P], F32)
        rs1_in = [dp.tile([CHS, H], COLL_DT, name=f"rs1_in{c}")
                  for c in range(NCH)]
        rs1_out = dp.tile([NCH, SHR, H], COLL_DT)
        ag_in = dp.tile([NCH, H, SHR], BF)
        ag_out = [dp.tile([8 * H, SHR], BF, addr_space="Shared", name=f"ag_out{c}")
                  for c in range(NCH)]
        rs2_in = [dp.tile([CHS, H], COLL_DT, name=f"rs2_in{c}")
                  for c in range(NCH)]
        rs2_out = dp.tile([NCH, SHR, H], COLL_DT)

        # hi/lo fp8 hidden (pre-rsq, ln1-scaled), lives phase 1-2
        ht_cm = tc.tile_pool(name="ht", bufs=1)
        htp = ht_cm.__enter__()
        ht_hi = htp.tile([P, NKC, S], F8)
        ht_lo = htp.tile([P, NKC, S], F8L)

        # ==== phase 1: hiddenT load, sumsq, ln1-scale + hi/lo split ====
        with tc.tile_pool(name="hin", bufs=3) as hinp, \
             tc.tile_pool(name="sqp", bufs=2) as sqp, \
             tc.tile_pool(name="ssps", bufs=1, space="PSUM") as ssp, \
             tc.tile_pool(name="smal", bufs=1) as smp:
            ss = ssp.tile([1, S], F32)
            for kp in range(NKP):
                hfa = hinp.tile([P, S], BF, tag="hf")
                nc.sync.dma_start(hfa[:], hiddent[(2 * kp) * P:(2 * kp + 1) * P, :])
                hfb = hinp.tile([P, S], BF, tag="hf")
                nc.sync.dma_start(hfb[:], hiddent[(2 * kp + 1) * P:(2 * kp + 2) * P, :])
                # baseline bf16 sumsq: [1,512] chains are full 2KB psum
                # zero-regions, so each chain's start zeroes only itself
                # (sub-bank DoubleRow chains wiped their bank neighbors)
                for hfx in (hfa, hfb):
                    sq = sqp.tile([P, S], BF, tag="sq")
                    nc.scalar.activation(sq[:], hfx[:], AF.Square)
                    for j in range(4):
                        nc.tensor.matmul(
                            ss[:, j * 512:(j + 1) * 512], ones_bf[:],
                            sq[:, j * 512:(j + 1) * 512],
                            start=(kp == 0 and hfx is hfa),
                            stop=(kp == NKP - 1 and hfx is hfb))
                for k2, hf in ((0, hfa), (1, hfb)):
                    k = 2 * kp + k2
                    tmp = sqp.tile([P, S], BF, tag="tmp")
                    nc.vector.tensor_tensor(
                        tmp[:], hf[:],
                        ln1sb[:, k:k + 1].to_broadcast((P, S)), MUL)
                    nc.scalar.copy(ht_hi[:, k, :], tmp[:])
                    nc.vector.tensor_tensor(
                        ht_lo[:, k, :], tmp[:], ht_hi[:, k, :], SUB)
            # rsq' = 1/(WS*std) so the q/k/v epilogue also undoes the WS
            # weight prescale: sqrt(WS^2*(ss/H + eps)) with WS^2/H == 1
            wseps = smp.tile([P, 1], F32, tag="wseps")
            nc.vector.memset(wseps[:], WS * WS * EPS)
            std = smp.tile([1, S], F32, tag="std")
            nc.scalar.activation(std[:], ss[:], AF.Sqrt,
                                 bias=wseps[:1, :], scale=WS * WS / H)
            rsq = smp.tile([1, S], F32, tag="rsq")
            nc.vector.reciprocal(rsq[:], std[:])
            nc.sync.dma_start(rsq_d[:], rsq[:])
            nc.sync.dma_start(
                rsq_pcol[:], rsq_d.rearrange("o (n p) -> p (o n)", p=P))

        # right side: qkv outputs, live to end of attention
        qkv_cm = tc.tile_pool(name="qkv", bufs=1, side="right")
        qkvp = qkv_cm.__enter__()
        qT = qkvp.tile([P, NH_LOC, S], BF)
        kT = qkvp.tile([P, NH_LOC, S], BF)
        v_sb = qkvp.tile([P, NST, QKV_LOC], BF)
        rsq_bc = qkvp.tile([P, S], F32)
        nc.gpsimd.dma_start(rsq_bc[:], rsq_d[:].to_broadcast((P, S)))

        # ==== phase 2: QKV projections (split-fp8 DoubleRow) ====
        with tc.tile_pool(name="wst", bufs=2) as wsp, \
             tc.tile_pool(name="qps", bufs=1, space="PSUM") as qpsp:
            for part in range(2):       # 0 = q, 1 = k
                dst = qT if part == 0 else kT
                for h in range(NH_LOC):
                    whi = wsp.tile([P, NKC, DH], F8, tag="whi")
                    nc.gpsimd.dma_start(whi[:], wqk8h[part, h])
                    wlo = wsp.tile([P, NKC, DH], F8L, tag="wlo")
                    nc.gpsimd.dma_start(wlo[:], wqk8l[part, h])
                    ps = [qpsp.tile([P, 512], F32, tag=f"qk{j}", name=f"qk{j}")
                          for j in range(4)]
                    for kp in range(NKP):
                        sl = slice(2 * kp, 2 * kp + 2)
                        for j in range(4):
                            o = j * 512
                            for w_t, x_t in ((whi, ht_hi), (wlo, ht_hi),
                                             (whi, ht_lo)):
                                nc.tensor.matmul(
                                    ps[j][:], w_t[:, sl, :],
                                    x_t[:, sl, o:o + 512],
                                    start=(kp == 0 and w_t is whi
                                           and x_t is ht_hi),
                                    stop=(kp == NKP - 1 and x_t is ht_lo),
                                    perf_mode=DR)
                    for j in range(4):
                        nc.vector.tensor_tensor(
                            dst[:, h, j * 512:(j + 1) * 512], ps[j][:],
                            rsq_bc[:, j * 512:(j + 1) * 512], MUL)
            # v in natural [s, d] layout via lhsT = ht pair chunks
            for vg in range(2):
                ps = [qpsp.tile([P, 512], F32, tag=f"qk{j}", name=f"v{j}")
                      for j in range(4)] + \
                     [qpsp.tile([P, 512], F32, tag=f"v{j}", name=f"v{j+4}")
                      for j in range(4)]
                for kp in range(NKP):
                    sl = slice(2 * kp, 2 * kp + 2)
                    vhi = wsp.tile([P, 2, QKV_LOC], F8, tag="vhi")
                    nc.gpsimd.dma_start(vhi[:], wv8h[kp])
                    vlo = wsp.tile([P, 2, QKV_LOC], F8L, tag="vlo")
                    nc.gpsimd.dma_start(vlo[:], wv8l[kp])
                    for sti in range(8):
                        st = vg * 8 + sti
                        for w_t, x_t in ((vhi, ht_hi), (vlo, ht_hi),
                                         (vhi, ht_lo)):
                            nc.tensor.matmul(
                                ps[sti][:],
                                x_t[:, sl, st * P:(st + 1) * P],
                                w_t[:],
                                start=(kp == 0 and w_t is vhi
                                       and x_t is ht_hi),
                                stop=(kp == NKP - 1 and x_t is ht_lo),
                                perf_mode=DR)
                for sti in range(8):
                    st = vg * 8 + sti
                    nc.scalar.activation(
                        v_sb[:, st, :], ps[sti][:], AF.Copy,
                        scale=rsq_pcol[:, st:st + 1])

        ht_cm.__exit__(None, None, None)   # free 96KB/part

        # o_proj weights (hi/lo) prefetched during attention
        opj_cm = tc.tile_pool(name="opj", bufs=1)
        opp = opj_cm.__enter__()
        ow_hi = opp.tile([P, NH_LOC, H], F8)
        ow_lo = opp.tile([P, NH_LOC, H], F8L)
        for h in range(NH_LOC):
            nc.gpsimd.dma_start(ow_hi[:, h, :], ow8h[:, h, :])
            nc.gpsimd.dma_start(ow_lo[:, h, :], ow8l[:, h, :])

        # attention output, hi/lo fp8, lives through o_proj
        atp_cm = tc.tile_pool(name="atp", bufs=1)
        atp = atp_cm.__enter__()
        at_hi = atp.tile([P, NH_LOC, S], F8)
        at_lo = atp.tile([P, NH_LOC, S], F8L)

        # ==== phase 3: attention (bf16, as baseline) ====
        with tc.tile_pool(name="msk", bufs=1) as mkp, \
             tc.tile_pool(name="probs", bufs=6) as prp, \
             tc.tile_pool(name="scps", bufs=2, space="PSUM") as scp, \
             tc.tile_pool(name="atps", bufs=1, space="PSUM") as apsp, \
             tc.tile_pool(name="attmisc", bufs=2) as amp:
            masksb = mkp.tile([P, NST, P], F32)
            nc.sync.dma_start(masksb[:], maskt.rearrange("n k q -> k n q"))
            for h in range(NH_LOC):
                aps = apsp.tile([P, S], F32, tag="aps", name="aps")
                sps = apsp.tile([P, NST], F32, tag="sps", name="sps")
                for kb in range(NST):
                    q0 = kb * P
                    pt = prp.tile([P, S], BF, tag="probs", name="pt")
                    bnds = []
                    a = q0
                    while a < S:
                        b = min((a // 512 + 1) * 512, S)
                        bnds.append((a, b))
                        a = b
                    for (a, b) in bnds:
                        sc = scp.tile([P, 512], F32, tag="sc", name="sc")
                        nc.tensor.matmul(
                            sc[:, :b - a], kT[:, h, q0:q0 + P],
                            qT[:, h, a:b], start=True, stop=True)
                        if a == q0:
                            nc.vector.tensor_tensor(
                                sc[:, :P], sc[:, :P], masksb[:, kb, :], ADD)
                        nc.scalar.activation(
                            pt[:, a:b], sc[:, :b - a], AF.Exp, scale=SCALE)
                    for (a, b) in bnds:
                        nc.tensor.matmul(
                            aps[:, a:b], v_sb[:, kb, h * DH:(h + 1) * DH],
                            pt[:, a:b],
                            start=(kb == 0), stop=(kb == (b - 1) // P))
                    for qb in range(kb, NST):
                        nc.tensor.matmul(
                            sps[:, qb:qb + 1], pt[:, qb * P:(qb + 1) * P],
                            ones_bf[:], start=(kb == 0 and qb == 0),
                            stop=(kb == qb), skip_group_check=True)
                rec = amp.tile([P, NST], F32, tag="rec")
                nc.vector.reciprocal(rec[:], sps[:])
                rtp = apsp.tile([NST, P], F32, tag="rtp", name="rtp")
                nc.tensor.transpose(rtp[:], rec[:], ident[:])
                rts = amp.tile([NST, P], F32, tag="rts")
                nc.scalar.copy(rts[:], rtp[:])
                nc.sync.dma_start(rec_d[h], rts[:])
                rbc = amp.tile([P, S], F32, tag="rbc")
                nc.gpsimd.dma_start(
                    rbc[:],
                    rec_d[h].rearrange("a b -> (a b)")[None, :]
                    .to_broadcast((P, S)))
                atb = amp.tile([P, S], BF, tag="atb")
                nc.vector.tensor_tensor(atb[:], aps[:], rbc[:], MUL)
                nc.scalar.copy(at_hi[:, h, :], atb[:])
                nc.vector.tensor_tensor(at_lo[:, h, :], atb[:],
                                        at_hi[:, h, :], SUB)

        qkv_cm.__exit__(None, None, None)

        # residual stream shards, live to the end (right side)
        h2_cm = tc.tile_pool(name="h2", bufs=1, side="right")
        h2p = h2_cm.__enter__()
        h2pk = [h2p.tile([P, H], F32, tag=f"h2_{j}", name=f"h2_{j}")
                for j in range(NCH // 2)]

        def h2sl(c):
            return h2pk[c // 2][(c % 2) * SHR:(c % 2) * SHR + SHR, :]

        # ==== phase 4: o_proj (split-fp8 DR) + per-chunk [RS1 -> norm -> AG] ====
        with tc.tile_pool(name="ops", bufs=1, space="PSUM") as opsp, \
             tc.tile_pool(name="ost", bufs=3) as ostp, \
             tc.tile_pool(name="tps", bufs=2, space="PSUM") as tpsp, \
             tc.tile_pool(name="chk", bufs=1) as chp:
            ln2bc = chp.tile([P, H], BF, tag="ln2bc")
            nc.gpsimd.dma_start(ln2bc[:], ln2[:].to_broadcast((P, H)))
            for st in range(NST):
                tsl = slice(st * P, (st + 1) * P)
                osb = ostp.tile([P, H], COLL_DT, tag="osb")
                for jh in range(2):      # H halves: 4 psum banks each
                    ps4 = [opsp.tile([P, 512], F32, tag=f"o{j}", name=f"o{j}")
                           for j in range(4)]
                    for hp in range(2):  # head pairs
                        sl = slice(2 * hp, 2 * hp + 2)
                        for j in range(4):
                            o = jh * 2048 + j * 512
                            for w_t, x_t in ((ow_hi, at_hi),
                                             (ow_lo, at_hi),
                                             (ow_hi, at_lo)):
                                nc.tensor.matmul(
                                    ps4[j][:],
                                    x_t[:, sl, tsl],
                                    w_t[:, sl, o:o + 512],
                                    start=(hp == 0 and w_t is ow_hi
                                           and x_t is at_hi),
                                    stop=(hp == 1 and x_t is at_lo),
                                    perf_mode=DR)
                    for j in range(4):
                        o = jh * 2048 + j * 512
                        if j % 2 == 0:
                            nc.vector.tensor_scalar_mul(
                                osb[:, o:o + 512], ps4[j][:], 1.0 / WS)
                        else:
                            nc.scalar.activation(
                                osb[:, o:o + 512], ps4[j][:],
                                AF.Copy, scale=1.0 / WS)
                nc.sync.dma_start(
                    rs1_in[st // 4][(st % 4) * P:(st % 4 + 1) * P, :], osb[:])
                if st % 4 != 3:
                    continue
                # chunk complete: ReduceScatter, then residual + rmsnorm +
                # transpose + AllGather inline so AG_c starts while o_proj of
                # later chunks still runs
                c = st // 4
                nc.gpsimd.collective_compute(
                    "ReduceScatter", ADD, replica_groups=RG,
                    ins=[rs1_in[c][:].opt()],
                    outs=[rs1_out[c].opt()])
                b = (c % 2) * SHR
                h2c = h2sl(c)
                nc.sync.dma_start(h2c, hidshard[c])
                tmp = chp.tile([P, H], COLL_DT, tag="tmp")
                nc.sync.dma_start(tmp[b:b + SHR, :], rs1_out[c])
                nc.vector.tensor_tensor(h2c, h2c, tmp[b:b + SHR, :], ADD)
                # baseline-proven natural-layout norm + DMA transpose
                sq2 = chp.tile([P, H], BF, tag="msh", name="sq2")
                nc.scalar.activation(sq2[b:b + SHR, :], h2c, AF.Square)
                var = chp.tile([P, 1], F32, tag="var")
                nc.vector.reduce_sum(var[b:b + SHR, :], sq2[b:b + SHR, :],
                                     axis=mybir.AxisListType.X)
                std2 = chp.tile([P, 1], F32, tag="std2")
                nc.scalar.activation(std2[b:b + SHR, :], var[b:b + SHR, :],
                                     AF.Sqrt, bias=epssb[b:b + SHR, :],
                                     scale=1.0 / H)
                rst = chp.tile([P, 1], F32, tag="rst")
                nc.vector.reciprocal(rst[b:b + SHR, :], std2[b:b + SHR, :])
                mtm = chp.tile([P, H], BF, tag="mtm")
                nc.scalar.activation(mtm[b:b + SHR, :], h2c, AF.Copy,
                                     scale=rst[b:b + SHR, :])
                msh = chp.tile([P, H], BF, tag="msh")
                nc.vector.tensor_tensor(msh[b:b + SHR, :], mtm[b:b + SHR, :],
                                        ln2bc[b:b + SHR, :], MUL)
                mts = chp.tile([P, NKC, SHR], BF, tag="mts")
                nc.sync.dma_start_transpose(mts[:], msh[b:b + SHR, :])
                nc.sync.dma_start(
                    ag_in[c].rearrange("(ks p) n -> p ks n", p=P), mts[:])
                nc.gpsimd.collective_compute(
                    "AllGather", mybir.AluOpType.bypass, replica_groups=RG,
                    ins=[ag_in[c].opt()], outs=[ag_out[c].opt()])

        atp_cm.__exit__(None, None, None)
        opj_cm.__exit__(None, None, None)

        # ==== phase 6+7: MLP per chunk (split-fp8 DR); weights stream per
        # chunk so each chunk's matmuls start right after its own AllGather
        # (an in-order PE queue would otherwise stall chunk c on AG_{c+1})
        import bass_rust as _br
        prev_mm = None
        for cpair in range(NCH):
            cs = [cpair]
            mh_cm = tc.tile_pool(name=f"mh{cpair}", bufs=1)
            mhp = mh_cm.__enter__()
            m_hi = {c: mhp.tile([P, NKC, CHS], F8, name=f"mhi{c}") for c in cs}
            m_lo = {c: mhp.tile([P, NKC, CHS], F8L, name=f"mlo{c}") for c in cs}
            had_hi = {c: mhp.tile([P, NIT, CHS], F8, name=f"hhi{c}") for c in cs}
            had_lo = {c: mhp.tile([P, NIT, CHS], F8L, name=f"hlo{c}") for c in cs}

            with tc.tile_pool(name="mtl", bufs=2) as mtlp:
                for c in cs:
                    mTb = mtlp.tile([P, NKC, CHS], BF, tag="mtb")
                    for r in range(8):
                        gi = nc.sync.dma_start(
                            mTb[:, :, r * SHR:(r + 1) * SHR],
                            ag_out[c][r * H:(r + 1) * H, :]
                            .rearrange("(ks p) n -> p ks n", p=P))
                        if prev_mm is not None:
                            _br.add_dep_helper(
                                gi.ins, prev_mm.ins, sync=False,
                                reason="order gathers after prev mlp")
                    # split in k-quarters so gate matmuls can start early
                    for kq in range(4):
                        ksl = slice(kq * 8, (kq + 1) * 8)
                        nc.scalar.copy(m_hi[c][:, ksl, :], mTb[:, ksl, :])
                        nc.vector.tensor_tensor(m_lo[c][:, ksl, :],
                                                mTb[:, ksl, :],
                                                m_hi[c][:, ksl, :], SUB)

            # gate/up, i-outer so weights stream once per chunk pair
            with tc.tile_pool(name="gst", bufs=2) as gsp, \
                 tc.tile_pool(name="gwa", bufs=2) as gwap, \
                 tc.tile_pool(name="gps", bufs=1, space="PSUM") as gpsp:
                for i in range(NIT):
                    ghi = gwap.tile([P, NKC, P], F8, tag="ghi")
                    nc.gpsimd.dma_start(ghi[:], g8h[i])
                    glo = gwap.tile([P, NKC, P], F8L, tag="glo")
                    nc.gpsimd.dma_start(glo[:], g8l[i])
                    uhi = gwap.tile([P, NKC, P], F8, tag="uhi")
                    nc.gpsimd.dma_start(uhi[:], u8h[i])
                    ulo = gwap.tile([P, NKC, P], F8L, tag="ulo")
                    nc.gpsimd.dma_start(ulo[:], u8l[i])
                    for c in cs:
                        gp = gpsp.tile([P, CHS], F32, tag=f"g{(2 * i + c) % 4}",
                                       name="gp")
                        up = gpsp.tile([P, CHS], F32, tag=f"u{(2 * i + c) % 4}",
                                       name="up")
                        for kp in range(NKP):
                            sl = slice(2 * kp, 2 * kp + 2)
                            for w_t, x_t in ((ghi, m_hi), (glo, m_hi),
                                             (ghi, m_lo)):
                                mm = nc.tensor.matmul(
                                    gp[:], w_t[:, sl, :],
                                    x_t[c][:, sl, :],
                                    start=(kp == 0 and w_t is ghi
                                           and x_t is m_hi),
                                    stop=(kp == NKP - 1 and x_t is m_lo),
                                    perf_mode=DR)
                                if prev_mm is None and i == 0:
                                    prev_mm = mm
                            for w_t, x_t in ((uhi, m_hi), (ulo, m_hi),
                                             (uhi, m_lo)):
                                nc.tensor.matmul(
                                    up[:], w_t[:, sl, :],
                                    x_t[c][:, sl, :],
                                    start=(kp == 0 and w_t is uhi
                                           and x_t is m_hi),
                                    stop=(kp == NKP - 1 and x_t is m_lo),
                                    perf_mode=DR)
                        gs = gsp.tile([P, CHS], BF, tag="gs")
                        us = gsp.tile([P, CHS], BF, tag="us")
                        nc.scalar.activation(gs[:], gp[:], AF.Silu,
                                             scale=1.0 / WS)
                        nc.vector.tensor_scalar_mul(us[:], up[:], 1.0 / WS)
                        hadt = gsp.tile([P, CHS], BF, tag="hadt")
                        nc.vector.tensor_tensor(hadt[:], gs[:], us[:], MUL)
                        nc.scalar.copy(had_hi[c][:, i, :], hadt[:])
                        nc.vector.tensor_tensor(had_lo[c][:, i, :], hadt[:],
                                                had_hi[c][:, i, :], SUB)

            # down proj, j-outer streams dw once per chunk pair
            with tc.tile_pool(name="dwp", bufs=2) as dwp, \
                 tc.tile_pool(name="dst", bufs=4) as dsp, \
                 tc.tile_pool(name="dps", bufs=2, space="PSUM") as dpsp:
                for j in range(8):       # 512-wide hid column blocks
                    dhi = dwp.tile([P, NIT, 512], F8, tag="dhi")
                    nc.gpsimd.dma_start(dhi[:], d8h[j])
                    dlo = dwp.tile([P, NIT, 512], F8L, tag="dlo")
                    nc.gpsimd.dma_start(dlo[:], d8l[j])
                    for c in cs:
                        for sti in range(4):
                            tsl = slice(sti * P, (sti + 1) * P)
                            dp_ = dpsp.tile([P, 512], F32,
                                            tag=f"d{sti}", name="dp")
                            for ip in range(NIP):
                                sl = slice(2 * ip, 2 * ip + 2)
                                for w_t, x_t in ((dhi, had_hi), (dlo, had_hi),
                                                 (dhi, had_lo)):
                                    nc.tensor.matmul(
                                        dp_[:], x_t[c][:, sl, tsl],
                                        w_t[:, sl, :],
                                        start=(ip == 0 and w_t is dhi
                                               and x_t is had_hi),
                                        stop=(ip == NIP - 1 and x_t is had_lo),
                                        perf_mode=DR)
                            dsb = dsp.tile([P, 512], COLL_DT, tag="dsb")
                            if (j + c) % 2 == 0:
                                nc.vector.tensor_scalar_mul(
                                    dsb[:], dp_[:], 1.0 / WS)
                            else:
                                nc.scalar.activation(
                                    dsb[:], dp_[:], AF.Copy, scale=1.0 / WS)
                            nc.sync.dma_start(
                                rs2_in[c][sti * P:(sti + 1) * P,
                                          j * 512:(j + 1) * 512], dsb[:])
                for c in cs:
                    nc.gpsimd.collective_compute(
                        "ReduceScatter", ADD, replica_groups=RG,
                        ins=[rs2_in[c][:].opt()],
                        outs=[rs2_out[c].opt()])
            mh_cm.__exit__(None, None, None)

        # ==== phase 8: final residual ====
        with tc.tile_pool(name="fin", bufs=1) as fpp:
            for c in range(NCH):
                b = (c % 2) * SHR
                f1 = fpp.tile([P, H], COLL_DT, tag="f1")
                nc.sync.dma_start(f1[b:b + SHR, :], rs2_out[c])
                fo = fpp.tile([P, H], F32, tag="fo")
                nc.vector.tensor_tensor(fo[b:b + SHR, :], f1[b:b + SHR, :],
                                        h2sl(c), ADD)
                nc.sync.dma_start(out[c], fo[b:b + SHR, :])

        h2_cm.__exit__(None, None, None)

    nc.finalize()
    return nc


def _split8(w):
    """Return (hi, lo) e4m3 fp8 split of float32 array w."""
    hi = w.astype(ml_dtypes.float8_e4m3)
    lo = (w - hi.astype(np.float32)).astype(ml_dtypes.float8_e5m2)
    return hi, lo


def _prep_inputs(hidden_states, attention_mask, W_pack, o_proj, gate_w, up_w,
                 down_w, ln1_w, ln2_w):
    """Slice/layout full inputs into 8 per-core input dicts."""
    hs = np.ascontiguousarray(np.asarray(hidden_states, dtype=np.float32)[0])
    hiddent = np.ascontiguousarray(hs.T).astype(ml_dtypes.bfloat16)  # [H, S]
    mask = np.asarray(attention_mask, dtype=np.float32)[0, 0]
    masktd = np.stack([
        np.ascontiguousarray(mask[b * P:(b + 1) * P, b * P:(b + 1) * P].T)
        for b in range(NST)])                                  # [NST, P, P]
    W_pack = np.asarray(W_pack, dtype=np.float32) * WS
    o_proj = np.asarray(o_proj, dtype=np.float32) * WS
    gate_w = np.asarray(gate_w, dtype=np.float32) * WS
    up_w = np.asarray(up_w, dtype=np.float32) * WS
    down_w = np.asarray(down_w, dtype=np.float32) * WS
    ln1 = np.ascontiguousarray(
        np.asarray(ln1_w, dtype=np.float32).reshape(NKC, P).T)  # [P, NKC]
    ln2 = np.asarray(ln2_w, dtype=np.float32).reshape(1, H)

    def to_pkc(w, cols):
        """[H, cols] -> [P, NKC, cols] with p the within-chunk row."""
        return np.ascontiguousarray(
            w.reshape(NKC, P, cols).transpose(1, 0, 2))

    in_maps = []
    for r in range(8):
        q0 = r * QKV_LOC
        # wqk8 [2, NH_LOC, P, NKC, DH] hi/lo
        wqkh = np.empty((2, NH_LOC, P, NKC, DH), ml_dtypes.float8_e4m3)
        wqkl = np.empty((2, NH_LOC, P, NKC, DH), ml_dtypes.float8_e5m2)
        for part in range(2):
            base = part * H + q0
            for h in range(NH_LOC):
                w = to_pkc(W_pack[:, base + h * DH: base + (h + 1) * DH], DH)
                wqkh[part, h], wqkl[part, h] = _split8(w)
        # wv8 [NKP, P, 2, QKV_LOC]
        wv = to_pkc(W_pack[:, 2 * H + q0: 2 * H + q0 + QKV_LOC], QKV_LOC)
        wv = np.ascontiguousarray(
            wv.reshape(P, NKP, 2, QKV_LOC).transpose(1, 0, 2, 3))
        wvh, wvl = _split8(wv)
        # ow8 [P, NH_LOC, H]: o_proj rows q0..q0+512 -> [h][p] -> [p][h]
        owr = np.ascontiguousarray(
            o_proj[q0:q0 + QKV_LOC, :].reshape(NH_LOC, P, H).transpose(1, 0, 2))
        owh, owl = _split8(owr)
        # gate/up columns [H, I_REAL] padded to I_LOC, layout [NIT, P, NKC, P]
        io0 = r * I_REAL
        gl = np.zeros((H, I_LOC), np.float32)
        gl[:, :I_REAL] = gate_w[:, io0:io0 + I_REAL]
        ul = np.zeros((H, I_LOC), np.float32)
        ul[:, :I_REAL] = up_w[:, io0:io0 + I_REAL]
        gv = np.stack([to_pkc(gl[:, i * P:(i + 1) * P], P) for i in range(NIT)])
        uv = np.stack([to_pkc(ul[:, i * P:(i + 1) * P], P) for i in range(NIT)])
        ghi, glo = _split8(gv)
        uhi, ulo = _split8(uv)
        # down rows [I_REAL, H] padded, layout [16, P, NIT, 256]
        dl = np.zeros((I_LOC, H), np.float32)
        dl[:I_REAL, :] = down_w[io0:io0 + I_REAL, :]
        dv = dl.reshape(NIT, P, H).transpose(1, 0, 2)     # [P, NIT, H]
        dv = np.ascontiguousarray(
            dv.reshape(P, NIT, 8, 512).transpose(2, 0, 1, 3))  # [8,P,NIT,512]
        dhi, dlo = _split8(dv)

        hsh = np.stack([
            hs[c * CHS + r * SHR: c * CHS + (r + 1) * SHR, :]
            for c in range(NCH)])                              # [NCH, SHR, H]
        in_maps.append({
            "hiddent": hiddent,
            "hidshard": np.ascontiguousarray(hsh),
            "maskt": masktd,
            "wqk8h": wqkh, "wqk8l": wqkl,
            "wv8h": wvh, "wv8l": wvl,
            "ow8h": owh, "ow8l": owl,
            "g8h": ghi, "g8l": glo,
            "u8h": uhi, "u8l": ulo,
            "d8h": dhi, "d8l": dlo,
            "ln1": ln1,
            "ln2": ln2,
        })
    return in_maps


def _assemble(results):
    """results[r]['out'] is [NCH, SHR, H]; reassemble [1, S, H]."""
    full = np.empty((S, H), np.float32)
    for r in range(8):
        o = results[r]["out"]
        for c in range(NCH):
            full[c * CHS + r * SHR: c * CHS + (r + 1) * SHR, :] = o[c]
    return full[None]


def _get_nc():
    if "nc" not in _CACHE:
        _CACHE["nc"] = _build()
    return _CACHE["nc"]


def kernel(**inputs):
    from concourse.bass_utils import run_bass_kernel_spmd
    nc = _get_nc()
    in_maps = _prep_inputs(**inputs)
    res = run_bass_kernel_spmd(nc, in_maps, core_ids=list(range(8)))
    return _assemble(res.results)


if __name__ == "__main__":
    rng = np.random.RandomState(0)
    ins = {
        "hidden_states": rng.randn(1, S, H).astype(np.float32),
        "attention_mask": np.where(
            np.tril(np.ones((S, S), bool)), 0.0,
            np.finfo(np.float32).min)[None, None].astype(np.float32),
        "W_pack": rng.randn(H, 3 * H).astype(np.float32) * 0.02,
        "o_proj": rng.randn(H, H).astype(np.float32) * 0.02,
        "gate_w": rng.randn(H, 11008).astype(np.float32) * 0.02,
        "up_w": rng.randn(H, 11008).astype(np.float32) * 0.02,
        "down_w": rng.randn(11008, H).astype(np.float32) * 0.02,
        "ln1_w": np.ones(H, np.float32),
        "ln2_w": np.ones(H, np.float32),
    }
    out = kernel(**ins)
    print("kernel output", out.shape, out.dtype, float(np.abs(out).mean()))


# revision 47
# speedup vs baseline: 1.1254x; 1.0628x over previous
"""Baichuan transformer layer on 8 Trainium2 NeuronCores, tensor-parallel.

Sharding: heads (32 -> 4/core) and MLP intermediate (11008 -> 1376/core,
padded to 1536) split across 8 cores. W_pack/gate/up sharded column-wise,
o_proj/down row-wise. bf16 ReduceScatter after o_proj (sequence-sharded
fp32 residual + RMSNorm), bf16 AllGather of the normed activations,
bf16 ReduceScatter after down_proj.

All heavy GEMMs (QKV, o_proj, gate/up, down) run as hi/lo-split fp8
DoubleRow matmuls: X @ W ~= Xhi@Whi + Xlo@Whi + Xhi@Wlo with hi parts in
e4m3 and lo parts in e5m2 (3 DoubleRow matmuls at 0.5 cyc/row replace 2
bf16 matmuls at 1 cyc/row per 256-deep contraction slice, at better-than-
bf16 accuracy). Attention (scores/softmax/PV) stays bf16. Weights are
pre-scaled by 64 on the host; the 1/64 descale folds into existing
epilogue scales.
"""

import math
import os
import sys

sys.path.insert(0, "/opt/trn_rl_repo")

import ml_dtypes
import numpy as np

import concourse.bass as bass
import concourse.tile as tile
from concourse import bacc, mybir
from concourse.masks import make_identity

P = 128
S = 2048
H = 4096
NKC = H // P            # 32 hidden chunks
NKP = NKC // 2          # 16 hidden pair-chunks
NH_LOC = 4              # heads per core
DH = 128
QKV_LOC = NH_LOC * DH   # 512
I_LOC = 1536            # padded local intermediate (12 * 128)
I_REAL = 11008 // 8     # 1376
NIT = I_LOC // P        # 12
NIP = NIT // 2          # 6
NST = S // P            # 16 seq tiles
NCH = 4                 # collective chunks
CHS = S // NCH          # 512 tokens per chunk
SHR = CHS // 8          # 64 rows per rank shard per chunk
EPS = 1e-6
SCALE = 1.0 / math.sqrt(DH)
WS = 64.0               # host-side weight scale before fp8 cast
BF = mybir.dt.bfloat16
F32 = mybir.dt.float32
F8 = mybir.dt.float8e4
# lo parts in e5m2: the ~2% split residuals are NORMAL-range in e5m2
# (min normal 2^-14) but subnormal in e4m3 (min normal 2^-6), and HW
# DoubleRow mishandles subnormal e4m3 inputs (e4m3-lo runs measured ~2x
# worse than e5m2-lo); e5m2 x e4m3 mixed DR probed bit-exact on HW
F8L = mybir.dt.float8e5

COLL_DT = mybir.dt.bfloat16
DR = mybir.MatmulPerfMode.DoubleRow

_CACHE = {}


def _build():
    nc = bacc.Bacc("TRN2", target_bir_lowering=False, debug=False, num_devices=8)

    hiddent = nc.dram_tensor("hiddent", [H, S], BF, kind="ExternalInput")
    hidshard = nc.dram_tensor("hidshard", [NCH, SHR, H], F32, kind="ExternalInput")
    maskt = nc.dram_tensor("maskt", [NST, P, P], F32, kind="ExternalInput")
    # [part(q/k), head, p, k, dh]
    wqk8h = nc.dram_tensor("wqk8h", [2, NH_LOC, P, NKC, DH], F8, kind="ExternalInput")
    wqk8l = nc.dram_tensor("wqk8l", [2, NH_LOC, P, NKC, DH], F8L, kind="ExternalInput")
    # [k-pair, p, 2, 512]
    wv8h = nc.dram_tensor("wv8h", [NKP, P, 2, QKV_LOC], F8, kind="ExternalInput")
    wv8l = nc.dram_tensor("wv8l", [NKP, P, 2, QKV_LOC], F8L, kind="ExternalInput")
    ow8h = nc.dram_tensor("ow8h", [P, NH_LOC, H], F8, kind="ExternalInput")
    ow8l = nc.dram_tensor("ow8l", [P, NH_LOC, H], F8L, kind="ExternalInput")
    g8h = nc.dram_tensor("g8h", [NIT, P, NKC, P], F8, kind="ExternalInput")
    g8l = nc.dram_tensor("g8l", [NIT, P, NKC, P], F8L, kind="ExternalInput")
    u8h = nc.dram_tensor("u8h", [NIT, P, NKC, P], F8, kind="ExternalInput")
    u8l = nc.dram_tensor("u8l", [NIT, P, NKC, P], F8L, kind="ExternalInput")
    d8h = nc.dram_tensor("d8h", [8, P, NIT, 512], F8, kind="ExternalInput")
    d8l = nc.dram_tensor("d8l", [8, P, NIT, 512], F8L, kind="ExternalInput")
    ln1 = nc.dram_tensor("ln1", [P, NKC], F32, kind="ExternalInput")
    ln2 = nc.dram_tensor("ln2", [1, H], F32, kind="ExternalInput")
    out = nc.dram_tensor("out", [NCH, SHR, H], F32, kind="ExternalOutput")

    RG = [list(range(8))]
    MUL = mybir.AluOpType.mult
    ADD = mybir.AluOpType.add
    SUB = mybir.AluOpType.subtract
    AF = mybir.ActivationFunctionType

    with tile.TileContext(nc) as tc:
      with tc.tile_pool(name="const", bufs=1) as cp, \
           tc.tile_pool(name="dram", bufs=1, space="DRAM") as dp:
        # ---- tiny constants ----
        ln1sb = cp.tile([P, NKC], F32)
        nc.sync.dma_start(ln1sb[:], ln1[:])
        ident = cp.tile([P, P], F32)
        make_identity(nc, ident[:])
        ident_bf = cp.tile([P, SHR], BF)
        make_identity(nc, ident_bf[:SHR, :])
        nc.sync.dma_start(ident_bf[SHR:2 * SHR, :], ident_bf[:SHR, :])
        ones_bf = cp.tile([P, 1], BF)
        nc.vector.memset(ones_bf[:], 1.0)
        # pair-step of the stationary AP must be even and 16B-aligned for
        # DoubleRow ldweights, so pad the ones column to 16 bytes
        ones8 = cp.tile([P, 2, 16], F8)
        nc.vector.memset(ones8[:], 1.0)
        epssb = cp.tile([P, 1], F32)
        nc.vector.memset(epssb[:], EPS)
        rsq_pcol = cp.tile([P, NST], F32)

        # dram scratch
        rsq_d = dp.tile([1, S], F32)
        rec_d = dp.tile([NH_LOC, NST, P], F32)
        rs1_in = [dp.tile([CHS, H], COLL_DT, name=f"rs1_in{c}")
                  for c in range(NCH)]
        rs1_out = dp.tile([NCH, SHR, H], COLL_DT)
        ag_in = dp.tile([NCH, H, SHR], BF)
        ag_out = [dp.tile([8 * H, SHR], BF, addr_space="Shared", name=f"ag_out{c}")
                  for c in range(NCH)]
        rs2_in = [dp.tile([CHS, H], COLL_DT, name=f"rs2_in{c}")
                  for c in range(NCH)]
        rs2_out = dp.tile([NCH, SHR, H], COLL_DT)

        # hi/lo fp8 hidden (pre-rsq, ln1-scaled), lives phase 1-2
        ht_cm = tc.tile_pool(name="ht", bufs=1)
        htp = ht_cm.__enter__()
        ht_hi = htp.tile([P, NKC, S], F8)
        ht_lo = htp.tile([P, NKC, S], F8L)

        # ==== phase 1: hiddenT load, sumsq, ln1-scale + hi/lo split ====
        with tc.tile_pool(name="hin", bufs=3) as hinp, \
             tc.tile_pool(name="sqp", bufs=2) as sqp, \
             tc.tile_pool(name="ssps", bufs=1, space="PSUM") as ssp, \
             tc.tile_pool(name="smal", bufs=1) as smp:
            ss = ssp.tile([1, S], F32)
            for kp in range(NKP):
                hfa = hinp.tile([P, S], BF, tag="hf")
                nc.sync.dma_start(hfa[:], hiddent[(2 * kp) * P:(2 * kp + 1) * P, :])
                hfb = hinp.tile([P, S], BF, tag="hf")
                nc.sync.dma_start(hfb[:], hiddent[(2 * kp + 1) * P:(2 * kp + 2) * P, :])
                # baseline bf16 sumsq: [1,512] chains are full 2KB psum
                # zero-regions, so each chain's start zeroes only itself
                # (sub-bank DoubleRow chains wiped their bank neighbors)
                for hfx in (hfa, hfb):
                    sq = sqp.tile([P, S], BF, tag="sq")
                    nc.scalar.activation(sq[:], hfx[:], AF.Square)
                    for j in range(4):
                        nc.tensor.matmul(
                            ss[:, j * 512:(j + 1) * 512], ones_bf[:],
                            sq[:, j * 512:(j + 1) * 512],
                            start=(kp == 0 and hfx is hfa),
                            stop=(kp == NKP - 1 and hfx is hfb))
                for k2, hf in ((0, hfa), (1, hfb)):
                    k = 2 * kp + k2
                    tmp = sqp.tile([P, S], BF, tag="tmp")
                    nc.vector.tensor_tensor(
                        tmp[:], hf[:],
                        ln1sb[:, k:k + 1].to_broadcast((P, S)), MUL)
                    nc.scalar.copy(ht_hi[:, k, :], tmp[:])
                    nc.vector.tensor_tensor(
                        ht_lo[:, k, :], tmp[:], ht_hi[:, k, :], SUB)
            # rsq' = 1/(WS*std) so the q/k/v epilogue also undoes the WS
            # weight prescale: sqrt(WS^2*(ss/H + eps)) with WS^2/H == 1
            wseps = smp.tile([P, 1], F32, tag="wseps")
            nc.vector.memset(wseps[:], WS * WS * EPS)
            std = smp.tile([1, S], F32, tag="std")
            nc.scalar.activation(std[:], ss[:], AF.Sqrt,
                                 bias=wseps[:1, :], scale=WS * WS / H)
            rsq = smp.tile([1, S], F32, tag="rsq")
            nc.vector.reciprocal(rsq[:], std[:])
            nc.sync.dma_start(rsq_d[:], rsq[:])
            nc.sync.dma_start(
                rsq_pcol[:], rsq_d.rearrange("o (n p) -> p (o n)", p=P))

        # right side: qkv outputs, live to end of attention
        qkv_cm = tc.tile_pool(name="qkv", bufs=1, side="right")
        qkvp = qkv_cm.__enter__()
        qT = qkvp.tile([P, NH_LOC, S], BF)
        kT = qkvp.tile([P, NH_LOC, S], BF)
        v_sb = qkvp.tile([P, NST, QKV_LOC], BF)
        rsq_bc = qkvp.tile([P, S], F32)
        nc.gpsimd.dma_start(rsq_bc[:], rsq_d[:].to_broadcast((P, S)))

        # ==== phase 2: QKV projections (split-fp8 DoubleRow) ====
        with tc.tile_pool(name="wst", bufs=2) as wsp, \
             tc.tile_pool(name="qps", bufs=1, space="PSUM") as qpsp:
            for part in range(2):       # 0 = q, 1 = k
                dst = qT if part == 0 else kT
                for h in range(NH_LOC):
                    whi = wsp.tile([P, NKC, DH], F8, tag="whi")
                    nc.gpsimd.dma_start(whi[:], wqk8h[part, h])
                    wlo = wsp.tile([P, NKC, DH], F8L, tag="wlo")
                    nc.gpsimd.dma_start(wlo[:], wqk8l[part, h])
                    ps = [qpsp.tile([P, 512], F32, tag=f"qk{j}", name=f"qk{j}")
                          for j in range(4)]
                    for kp in range(NKP):
                        sl = slice(2 * kp, 2 * kp + 2)
                        for j in range(4):
                            o = j * 512
                            for w_t, x_t in ((whi, ht_hi), (wlo, ht_hi),
                                             (whi, ht_lo)):
                                nc.tensor.matmul(
                                    ps[j][:], w_t[:, sl, :],
                                    x_t[:, sl, o:o + 512],
                                    start=(kp == 0 and w_t is whi
                                           and x_t is ht_hi),
                                    stop=(kp == NKP - 1 and x_t is ht_lo),
                                    perf_mode=DR)
                    for j in range(4):
                        nc.vector.tensor_tensor(
                            dst[:, h, j * 512:(j + 1) * 512], ps[j][:],
                            rsq_bc[:, j * 512:(j + 1) * 512], MUL)
            # v in natural [s, d] layout via lhsT = ht pair chunks
            for vg in range(2):
                ps = [qpsp.tile([P, 512], F32, tag=f"qk{j}", name=f"v{j}")
                      for j in range(4)] + \
                     [qpsp.tile([P, 512], F32, tag=f"v{j}", name=f"v{j+4}")
                      for j in range(4)]
                for kp in range(NKP):
                    sl = slice(2 * kp, 2 * kp + 2)
                    vhi = wsp.tile([P, 2, QKV_LOC], F8, tag="vhi")
                    nc.gpsimd.dma_start(vhi[:], wv8h[kp])
                    vlo = wsp.tile([P, 2, QKV_LOC], F8L, tag="vlo")
                    nc.gpsimd.dma_start(vlo[:], wv8l[kp])
                    for sti in range(8):
                        st = vg * 8 + sti
                        for w_t, x_t in ((vhi, ht_hi), (vlo, ht_hi),
                                         (vhi, ht_lo)):
                            nc.tensor.matmul(
                                ps[sti][:],
                                x_t[:, sl, st * P:(st + 1) * P],
                                w_t[:],
                                start=(kp == 0 and w_t is vhi
                                       and x_t is ht_hi),
                                stop=(kp == NKP - 1 and x_t is ht_lo),
                                perf_mode=DR)
                for sti in range(8):
                    st = vg * 8 + sti
                    nc.scalar.activation(
                        v_sb[:, st, :], ps[sti][:], AF.Copy,
                        scale=rsq_pcol[:, st:st + 1])

        ht_cm.__exit__(None, None, None)   # free 96KB/part

        # o_proj weights (hi/lo) prefetched during attention
        opj_cm = tc.tile_pool(name="opj", bufs=1)
        opp = opj_cm.__enter__()
        ow_hi = opp.tile([P, NH_LOC, H], F8)
        ow_lo = opp.tile([P, NH_LOC, H], F8L)
        for h in range(NH_LOC):
            nc.gpsimd.dma_start(ow_hi[:, h, :], ow8h[:, h, :])
            nc.gpsimd.dma_start(ow_lo[:, h, :], ow8l[:, h, :])

        # attention output, hi/lo fp8, lives through o_proj
        atp_cm = tc.tile_pool(name="atp", bufs=1)
        atp = atp_cm.__enter__()
        at_hi = atp.tile([P, NH_LOC, S], F8)
        at_lo = atp.tile([P, NH_LOC, S], F8L)

        # ==== phase 3: attention (bf16, as baseline) ====
        with tc.tile_pool(name="msk", bufs=1) as mkp, \
             tc.tile_pool(name="probs", bufs=6) as prp, \
             tc.tile_pool(name="scps", bufs=2, space="PSUM") as scp, \
             tc.tile_pool(name="atps", bufs=1, space="PSUM") as apsp, \
             tc.tile_pool(name="attmisc", bufs=2) as amp:
            masksb = mkp.tile([P, NST, P], F32)
            nc.sync.dma_start(masksb[:], maskt.rearrange("n k q -> k n q"))
            for h in range(NH_LOC):
                aps = apsp.tile([P, S], F32, tag="aps", name="aps")
                sps = apsp.tile([P, NST], F32, tag="sps", name="sps")
                for kb in range(NST):
                    q0 = kb * P
                    pt = prp.tile([P, S], BF, tag="probs", name="pt")
                    bnds = []
                    a = q0
                    while a < S:
                        b = min((a // 512 + 1) * 512, S)
                        bnds.append((a, b))
                        a = b
                    for (a, b) in bnds:
                        sc = scp.tile([P, 512], F32, tag="sc", name="sc")
                        nc.tensor.matmul(
                            sc[:, :b - a], kT[:, h, q0:q0 + P],
                            qT[:, h, a:b], start=True, stop=True)
                        if a == q0:
                            nc.vector.tensor_tensor(
                                sc[:, :P], sc[:, :P], masksb[:, kb, :], ADD)
                        nc.scalar.activation(
                            pt[:, a:b], sc[:, :b - a], AF.Exp, scale=SCALE)
                    for (a, b) in bnds:
                        nc.tensor.matmul(
                            aps[:, a:b], v_sb[:, kb, h * DH:(h + 1) * DH],
                            pt[:, a:b],
                            start=(kb == 0), stop=(kb == (b - 1) // P))
                    for qb in range(kb, NST):
                        nc.tensor.matmul(
                            sps[:, qb:qb + 1], pt[:, qb * P:(qb + 1) * P],
                            ones_bf[:], start=(kb == 0 and qb == 0),
                            stop=(kb == qb), skip_group_check=True)
                rec = amp.tile([P, NST], F32, tag="rec")
                nc.vector.reciprocal(rec[:], sps[:])
                rtp = apsp.tile([NST, P], F32, tag="rtp", name="rtp")
                nc.tensor.transpose(rtp[:], rec[:], ident[:])
                rts = amp.tile([NST, P], F32, tag="rts")
                nc.scalar.copy(rts[:], rtp[:])
                nc.sync.dma_start(rec_d[h], rts[:])
                rbc = amp.tile([P, S], F32, tag="rbc")
                nc.gpsimd.dma_start(
                    rbc[:],
                    rec_d[h].rearrange("a b -> (a b)")[None, :]
                    .to_broadcast((P, S)))
                atb = amp.tile([P, S], BF, tag="atb")
                nc.vector.tensor_tensor(atb[:], aps[:], rbc[:], MUL)
                nc.scalar.copy(at_hi[:, h, :], atb[:])
                nc.vector.tensor_tensor(at_lo[:, h, :], atb[:],
                                        at_hi[:, h, :], SUB)

        qkv_cm.__exit__(None, None, None)

        # residual stream shards, live to the end (right side)
        h2_cm = tc.tile_pool(name="h2", bufs=1, side="right")
        h2p = h2_cm.__enter__()
        h2pk = [h2p.tile([P, H], F32, tag=f"h2_{j}", name=f"h2_{j}")
                for j in range(NCH // 2)]

        def h2sl(c):
            return h2pk[c // 2][(c % 2) * SHR:(c % 2) * SHR + SHR, :]

        # ==== phase 4: o_proj (split-fp8 DR) + per-chunk [RS1 -> norm -> AG] ====
        with tc.tile_pool(name="ops", bufs=1, space="PSUM") as opsp, \
             tc.tile_pool(name="ost", bufs=3) as ostp, \
             tc.tile_pool(name="tps", bufs=2, space="PSUM") as tpsp, \
             tc.tile_pool(name="chk", bufs=1) as chp:
            ln2bc = chp.tile([P, H], BF, tag="ln2bc")
            nc.gpsimd.dma_start(ln2bc[:], ln2[:].to_broadcast((P, H)))
            for st in range(NST):
                tsl = slice(st * P, (st + 1) * P)
                osb = ostp.tile([P, H], COLL_DT, tag="osb")
                for jh in range(2):      # H halves: 4 psum banks each
                    ps4 = [opsp.tile([P, 512], F32, tag=f"o{j}", name=f"o{j}")
                           for j in range(4)]
                    for hp in range(2):  # head pairs
                        sl = slice(2 * hp, 2 * hp + 2)
                        for j in range(4):
                            o = jh * 2048 + j * 512
                            for w_t, x_t in ((ow_hi, at_hi),
                                             (ow_lo, at_hi),
                                             (ow_hi, at_lo)):
                                nc.tensor.matmul(
                                    ps4[j][:],
                                    x_t[:, sl, tsl],
                                    w_t[:, sl, o:o + 512],
                                    start=(hp == 0 and w_t is ow_hi
                                           and x_t is at_hi),
                                    stop=(hp == 1 and x_t is at_lo),
                                    perf_mode=DR)
                    for j in range(4):
                        o = jh * 2048 + j * 512
                        if j % 2 == 0:
                            nc.vector.tensor_scalar_mul(
                                osb[:, o:o + 512], ps4[j][:], 1.0 / WS)
                        else:
                            nc.scalar.activation(
                                osb[:, o:o + 512], ps4[j][:],
                                AF.Copy, scale=1.0 / WS)
                nc.sync.dma_start(
                    rs1_in[st // 4][(st % 4) * P:(st % 4 + 1) * P, :], osb[:])
                if st % 4 != 3:
                    continue
                # chunk complete: ReduceScatter, then residual + rmsnorm +
                # transpose + AllGather inline so AG_c starts while o_proj of
                # later chunks still runs
                c = st // 4
                nc.gpsimd.collective_compute(
                    "ReduceScatter", ADD, replica_groups=RG,
                    ins=[rs1_in[c][:].opt()],
                    outs=[rs1_out[c].opt()])
                b = (c % 2) * SHR
                h2c = h2sl(c)
                nc.sync.dma_start(h2c, hidshard[c])
                tmp = chp.tile([P, H], COLL_DT, tag="tmp")
                nc.sync.dma_start(tmp[b:b + SHR, :], rs1_out[c])
                nc.vector.tensor_tensor(h2c, h2c, tmp[b:b + SHR, :], ADD)
                # baseline-proven natural-layout norm + DMA transpose
                sq2 = chp.tile([P, H], BF, tag="msh", name="sq2")
                nc.scalar.activation(sq2[b:b + SHR, :], h2c, AF.Square)
                var = chp.tile([P, 1], F32, tag="var")
                nc.vector.reduce_sum(var[b:b + SHR, :], sq2[b:b + SHR, :],
                                     axis=mybir.AxisListType.X)
                std2 = chp.tile([P, 1], F32, tag="std2")
                nc.scalar.activation(std2[b:b + SHR, :], var[b:b + SHR, :],
                                     AF.Sqrt, bias=epssb[b:b + SHR, :],
                                     scale=1.0 / H)
                rst = chp.tile([P, 1], F32, tag="rst")
                nc.vector.reciprocal(rst[b:b + SHR, :], std2[b:b + SHR, :])
                mtm = chp.tile([P, H], BF, tag="mtm")
                nc.scalar.activation(mtm[b:b + SHR, :], h2c, AF.Copy,
                                     scale=rst[b:b + SHR, :])
                msh = chp.tile([P, H], BF, tag="msh")
                nc.vector.tensor_tensor(msh[b:b + SHR, :], mtm[b:b + SHR, :],
                                        ln2bc[b:b + SHR, :], MUL)
                mts = chp.tile([P, NKC, SHR], BF, tag="mts")
                nc.sync.dma_start_transpose(mts[:], msh[b:b + SHR, :])
                nc.sync.dma_start(
                    ag_in[c].rearrange("(ks p) n -> p ks n", p=P), mts[:])
                nc.gpsimd.collective_compute(
                    "AllGather", mybir.AluOpType.bypass, replica_groups=RG,
                    ins=[ag_in[c].opt()], outs=[ag_out[c].opt()])

        atp_cm.__exit__(None, None, None)
        opj_cm.__exit__(None, None, None)

        # ==== phase 6+7: MLP per chunk (split-fp8 DR); weights stream per
        # chunk so each chunk's matmuls start right after its own AllGather
        # (an in-order PE queue would otherwise stall chunk c on AG_{c+1})
        import bass_rust as _br
        prev_mm = None
        # m tiles double-buffered across chunks so chunk c+1's gather+split
        # overlaps chunk c's matmuls (per-chunk pools serialized ~55us/chunk
        # on SBUF reuse); had tiles single-buffered to stay within SBUF
        mh_cm = tc.tile_pool(name="mh", bufs=2)
        mhp = mh_cm.__enter__()
        hd_cm = tc.tile_pool(name="hd", bufs=1)
        hdp = hd_cm.__enter__()
        mt_cm2 = tc.tile_pool(name="mtl", bufs=2)
        mtlp = mt_cm2.__enter__()
        for cpair in range(NCH):
            cs = [cpair]
            m_hi = {c: mhp.tile([P, NKC, CHS], F8, tag="mhi", name=f"mhi{c}")
                    for c in cs}
            m_lo = {c: mhp.tile([P, NKC, CHS], F8L, tag="mlo", name=f"mlo{c}")
                    for c in cs}
            had_hi = {c: hdp.tile([P, NIT, CHS], F8, tag="hhi", name=f"hhi{c}")
                      for c in cs}
            had_lo = {c: hdp.tile([P, NIT, CHS], F8L, tag="hlo", name=f"hlo{c}")
                      for c in cs}

            for c in cs:
                for th in range(2):     # token halves keep the mTb tile small
                    hsl = slice(th * (CHS // 2), (th + 1) * (CHS // 2))
                    mTb = mtlp.tile([P, NKC, CHS // 2], BF, tag="mtb")
                    for rr in range(4):
                        r = th * 4 + rr
                        gi = nc.sync.dma_start(
                            mTb[:, :, rr * SHR:(rr + 1) * SHR],
                            ag_out[c][r * H:(r + 1) * H, :]
                            .rearrange("(ks p) n -> p ks n", p=P))
                        if prev_mm is not None:
                            _br.add_dep_helper(
                                gi.ins, prev_mm.ins, sync=False,
                                reason="order gathers after prev mlp")
                    # split in k-quarters so gate matmuls can start early
                    for kq in range(4):
                        ksl = slice(kq * 8, (kq + 1) * 8)
                        nc.scalar.copy(m_hi[c][:, ksl, hsl], mTb[:, ksl, :])
                        nc.vector.tensor_tensor(m_lo[c][:, ksl, hsl],
                                                mTb[:, ksl, :],
                                                m_hi[c][:, ksl, hsl], SUB)

            # gate/up, i-outer so weights stream once per chunk pair
            with tc.tile_pool(name="gst", bufs=2) as gsp, \
                 tc.tile_pool(name="gwa", bufs=2) as gwap, \
                 tc.tile_pool(name="gps", bufs=1, space="PSUM") as gpsp:
                for i in range(NIT):
                    ghi = gwap.tile([P, NKC, P], F8, tag="ghi")
                    nc.gpsimd.dma_start(ghi[:], g8h[i])
                    glo = gwap.tile([P, NKC, P], F8L, tag="glo")
                    nc.gpsimd.dma_start(glo[:], g8l[i])
                    uhi = gwap.tile([P, NKC, P], F8, tag="uhi")
                    nc.gpsimd.dma_start(uhi[:], u8h[i])
                    ulo = gwap.tile([P, NKC, P], F8L, tag="ulo")
                    nc.gpsimd.dma_start(ulo[:], u8l[i])
                    for c in cs:
                        gp = gpsp.tile([P, CHS], F32, tag=f"g{(2 * i + c) % 4}",
                                       name="gp")
                        up = gpsp.tile([P, CHS], F32, tag=f"u{(2 * i + c) % 4}",
                                       name="up")
                        for kp in range(NKP):
                            sl = slice(2 * kp, 2 * kp + 2)
                            for w_t, x_t in ((ghi, m_hi), (glo, m_hi),
                                             (ghi, m_lo)):
                                mm = nc.tensor.matmul(
                                    gp[:], w_t[:, sl, :],
                                    x_t[c][:, sl, :],
                                    start=(kp == 0 and w_t is ghi
                                           and x_t is m_hi),
                                    stop=(kp == NKP - 1 and x_t is m_lo),
                                    perf_mode=DR)
                                if prev_mm is None and i == 0:
                                    prev_mm = mm
                            for w_t, x_t in ((uhi, m_hi), (ulo, m_hi),
                                             (uhi, m_lo)):
                                nc.tensor.matmul(
                                    up[:], w_t[:, sl, :],
                                    x_t[c][:, sl, :],
                                    start=(kp == 0 and w_t is uhi
                                           and x_t is m_hi),
                                    stop=(kp == NKP - 1 and x_t is m_lo),
                                    perf_mode=DR)
                        gs = gsp.tile([P, CHS], BF, tag="gs")
                        us = gsp.tile([P, CHS], BF, tag="us")
                        nc.scalar.activation(gs[:], gp[:], AF.Silu,
                                             scale=1.0 / WS)
                        nc.vector.tensor_scalar_mul(us[:], up[:], 1.0 / WS)
                        hadt = gsp.tile([P, CHS], BF, tag="hadt")
                        nc.vector.tensor_tensor(hadt[:], gs[:], us[:], MUL)
                        nc.scalar.copy(had_hi[c][:, i, :], hadt[:])
                        nc.vector.tensor_tensor(had_lo[c][:, i, :], hadt[:],
                                                had_hi[c][:, i, :], SUB)

            # down proj, j-outer streams dw once per chunk pair
            with tc.tile_pool(name="dwp", bufs=2) as dwp, \
                 tc.tile_pool(name="dst", bufs=4) as dsp, \
                 tc.tile_pool(name="dps", bufs=2, space="PSUM") as dpsp:
                for j in range(8):       # 512-wide hid column blocks
                    dhi = dwp.tile([P, NIT, 512], F8, tag="dhi")
                    nc.gpsimd.dma_start(dhi[:], d8h[j])
                    dlo = dwp.tile([P, NIT, 512], F8L, tag="dlo")
                    nc.gpsimd.dma_start(dlo[:], d8l[j])
                    for c in cs:
                        for sti in range(4):
                            tsl = slice(sti * P, (sti + 1) * P)
                            dp_ = dpsp.tile([P, 512], F32,
                                            tag=f"d{sti}", name="dp")
                            for ip in range(NIP):
                                sl = slice(2 * ip, 2 * ip + 2)
                                for w_t, x_t in ((dhi, had_hi), (dlo, had_hi),
                                                 (dhi, had_lo)):
                                    nc.tensor.matmul(
                                        dp_[:], x_t[c][:, sl, tsl],
                                        w_t[:, sl, :],
                                        start=(ip == 0 and w_t is dhi
                                               and x_t is had_hi),
                                        stop=(ip == NIP - 1 and x_t is had_lo),
                                        perf_mode=DR)
                            dsb = dsp.tile([P, 512], COLL_DT, tag="dsb")
                            if (j + c) % 2 == 0:
                                nc.vector.tensor_scalar_mul(
                                    dsb[:], dp_[:], 1.0 / WS)
                            else:
                                nc.scalar.activation(
                                    dsb[:], dp_[:], AF.Copy, scale=1.0 / WS)
                            nc.sync.dma_start(
                                rs2_in[c][sti * P:(sti + 1) * P,
                                          j * 512:(j + 1) * 512], dsb[:])
                for c in cs:
                    nc.gpsimd.collective_compute(
                        "ReduceScatter", ADD, replica_groups=RG,
                        ins=[rs2_in[c][:].opt()],
                        outs=[rs2_out[c].opt()])
        mt_cm2.__exit__(None, None, None)
        hd_cm.__exit__(None, None, None)
        mh_cm.__exit__(None, None, None)

        # ==== phase 8: final residual ====
        with tc.tile_pool(name="fin", bufs=1) as fpp:
            for c in range(NCH):
                b = (c % 2) * SHR
                f1 = fpp.tile([P, H], COLL_DT, tag="f1")
                nc.sync.dma_start(f1[b:b + SHR, :], rs2_out[c])
                fo = fpp.tile([P, H], F32, tag="fo")
                nc.vector.tensor_tensor(fo[b:b + SHR, :], f1[b:b + SHR, :],
                                        h2sl(c), ADD)
                nc.sync.dma_start(out[c], fo[b:b + SHR, :])

        h2_cm.__exit__(None, None, None)

    nc.finalize()
    return nc


def _split8(w):
    """Return (hi, lo) e4m3 fp8 split of float32 array w."""
    hi = w.astype(ml_dtypes.float8_e4m3)
    lo = (w - hi.astype(np.float32)).astype(ml_dtypes.float8_e5m2)
    return hi, lo


def _prep_inputs(hidden_states, attention_mask, W_pack, o_proj, gate_w, up_w,
                 down_w, ln1_w, ln2_w):
    """Slice/layout full inputs into 8 per-core input dicts."""
    hs = np.ascontiguousarray(np.asarray(hidden_states, dtype=np.float32)[0])
    hiddent = np.ascontiguousarray(hs.T).astype(ml_dtypes.bfloat16)  # [H, S]
    mask = np.asarray(attention_mask, dtype=np.float32)[0, 0]
    masktd = np.stack([
        np.ascontiguousarray(mask[b * P:(b + 1) * P, b * P:(b + 1) * P].T)
        for b in range(NST)])                                  # [NST, P, P]
    W_pack = np.asarray(W_pack, dtype=np.float32) * WS
    o_proj = np.asarray(o_proj, dtype=np.float32) * WS
    gate_w = np.asarray(gate_w, dtype=np.float32) * WS
    up_w = np.asarray(up_w, dtype=np.float32) * WS
    down_w = np.asarray(down_w, dtype=np.float32) * WS
    ln1 = np.ascontiguousarray(
        np.asarray(ln1_w, dtype=np.float32).reshape(NKC, P).T)  # [P, NKC]
    ln2 = np.asarray(ln2_w, dtype=np.float32).reshape(1, H)

    def to_pkc(w, cols):
        """[H, cols] -> [P, NKC, cols] with p the within-chunk row."""
        return np.ascontiguousarray(
            w.reshape(NKC, P, cols).transpose(1, 0, 2))

    in_maps = []
    for r in range(8):
        q0 = r * QKV_LOC
        # wqk8 [2, NH_LOC, P, NKC, DH] hi/lo
        wqkh = np.empty((2, NH_LOC, P, NKC, DH), ml_dtypes.float8_e4m3)
        wqkl = np.empty((2, NH_LOC, P, NKC, DH), ml_dtypes.float8_e5m2)
        for part in range(2):
            base = part * H + q0
            for h in range(NH_LOC):
                w = to_pkc(W_pack[:, base + h * DH: base + (h + 1) * DH], DH)
                wqkh[part, h], wqkl[part, h] = _split8(w)
        # wv8 [NKP, P, 2, QKV_LOC]
        wv = to_pkc(W_pack[:, 2 * H + q0: 2 * H + q0 + QKV_LOC], QKV_LOC)
        wv = np.ascontiguousarray(
            wv.reshape(P, NKP, 2, QKV_LOC).transpose(1, 0, 2, 3))
        wvh, wvl = _split8(wv)
        # ow8 [P, NH_LOC, H]: o_proj rows q0..q0+512 -> [h][p] -> [p][h]
        owr = np.ascontiguousarray(
            o_proj[q0:q0 + QKV_LOC, :].reshape(NH_LOC, P, H).transpose(1, 0, 2))
        owh, owl = _split8(owr)
        # gate/up columns [H, I_REAL] padded to I_LOC, layout [NIT, P, NKC, P]
        io0 = r * I_REAL
        gl = np.zeros((H, I_LOC), np.float32)
        gl[:, :I_REAL] = gate_w[:, io0:io0 + I_REAL]
        ul = np.zeros((H, I_LOC), np.float32)
        ul[:, :I_REAL] = up_w[:, io0:io0 + I_REAL]
        gv = np.stack([to_pkc(gl[:, i * P:(i + 1) * P], P) for i in range(NIT)])
        uv = np.stack([to_pkc(ul[:, i * P:(i + 1) * P], P) for i in range(NIT)])
        ghi, glo = _split8(gv)
        uhi, ulo = _split8(uv)
        # down rows [I_REAL, H] padded, layout [16, P, NIT, 256]
        dl = np.zeros((I_LOC, H), np.float32)
        dl[:I_REAL, :] = down_w[io0:io0 + I_REAL, :]
        dv = dl.reshape(NIT, P, H).transpose(1, 0, 2)     # [P, NIT, H]
        dv = np.ascontiguousarray(
            dv.reshape(P, NIT, 8, 512).transpose(2, 0, 1, 3))  # [8,P,NIT,512]
        dhi, dlo = _split8(dv)

        hsh = np.stack([
            hs[c * CHS + r * SHR: c * CHS + (r + 1) * SHR, :]
            for c in range(NCH)])                              # [NCH, SHR, H]
        in_maps.append({
            "hiddent": hiddent,
            "hidshard": np.ascontiguousarray(hsh),
            "maskt": masktd,
            "wqk8h": wqkh, "wqk8l": wqkl,
            "wv8h": wvh, "wv8l": wvl,
            "ow8h": owh, "ow8l": owl,
            "g8h": ghi, "g8l": glo,
            "u8h": uhi, "u8l": ulo,
            "d8h": dhi, "d8l": dlo,
            "ln1": ln1,
            "ln2": ln2,
        })
    return in_maps


def _assemble(results):
    """results[r]['out'] is [NCH, SHR, H]; reassemble [1, S, H]."""
    full = np.empty((S, H), np.float32)
    for r in range(8):
        o = results[r]["out"]
        for c in range(NCH):
            full[c * CHS + r * SHR: c * CHS + (r + 1) * SHR, :] = o[c]
    return full[None]


def _get_nc():
    if "nc" not in _CACHE:
        _CACHE["nc"] = _build()
    return _CACHE["nc"]


def kernel(**inputs):
    from concourse.bass_utils import run_bass_kernel_spmd
    nc = _get_nc()
    in_maps = _prep_inputs(**inputs)
    res = run_bass_kernel_spmd(nc, in_maps, core_ids=list(range(8)))
    return _assemble(res.results)


if __name__ == "__main__":
    rng = np.random.RandomState(0)
    ins = {
        "hidden_states": rng.randn(1, S, H).astype(np.float32),
        "attention_mask": np.where(
            np.tril(np.ones((S, S), bool)), 0.0,
            np.finfo(np.float32).min)[None, None].astype(np.float32),
        "W_pack": rng.randn(H, 3 * H).astype(np.float32) * 0.02,
        "o_proj": rng.randn(H, H).astype(np.float32) * 0.02,
        "gate_w": rng.randn(H, 11008).astype(np.float32) * 0.02,
        "up_w": rng.randn(H, 11008).astype(np.float32) * 0.02,
        "down_w": rng.randn(11008, H).astype(np.float32) * 0.02,
        "ln1_w": np.ones(H, np.float32),
        "ln2_w": np.ones(H, np.float32),
    }
    out = kernel(**ins)
    print("kernel output", out.shape, out.dtype, float(np.abs(out).mean()))
